# revision 44
# baseline (speedup 1.0000x reference)
"""Multi-head causal attention (B=2,S=2048,D=1024,H=16,RoPE) on 8 TRN2 NeuronCores.

Sharding: core c handles batch b=c//4, head-group g=c%4 (4 heads each).
Wq/Wk/Wv column-sharded per head group, Wo row-sharded; the all-reduce over
head groups is realized as a host-side partial sum at gather time.

Per-core kernel, all matmul operands bf16 (fp32 PSUM accumulation):
  Projection round r: QKV projections for s-block r from pre-transposed x,
    Q/K kept feature-major [d, s], RoPE'd via partition-shift DMAs + DVE;
    V natural [s, d] with a ones column per head (softmax denominators ride
    the AV matmul).
  Attention (m, qb): per head-pair m the two heads' score matmuls are
    row-tiled (K=64 at partition bases 0/64) into one 2-bank PSUM pair tile
    and run concurrently; a single paired exp [128, 2, 512] on ACT covers
    both heads; binary diag-mask multiply on gpsimd; AV accumulation into
    [65, 512] PSUM per head.
  Weaving: projection round r+1 and output-projection round r-1 are emitted
    as fill pieces inside attention round r's kt loop so the ACT-bound
    attention stretches keep the PE busy.
  Normalize: puv evacuated to SBUF bf16 immediately (frees the PSUM bank),
    reciprocal of the denominator row on DVE, partition-broadcast via DMA on
    the scalar HWDGE ring, divide on DVE while writing vecT.
  Output projection: vecT @ Wo per 128-q chunk, bf16 partial out -> DRAM.
"""
import numpy as np
import ml_dtypes
from contextlib import ExitStack

import concourse.bass as bass
import concourse.tile as tile
from concourse import library_config, mybir
from concourse.bass_utils import run_bass_kernel_spmd

B, S, D, H, HD = 2, 2048, 1024, 16, 64
HPC = 4            # heads per core
DC = HPC * HD      # 256 features per core
NDT = D // 128     # 8 input-dim tiles
NST = S // 128     # 16 sequence/key tiles
NQB = S // 512     # 4 query blocks
MT = DC // 128     # 2 feature m-tiles for Q/K/vec

F32 = mybir.dt.float32
BF16 = mybir.dt.bfloat16
AF = mybir.ActivationFunctionType

_nop_ctr = [0]


def fix_engine_waits(nc, max_waits=1):
    """This walrus build rejects any engine instruction with >1 sync wait
    (single wait slot per instruction struct). Move excess waits onto
    same-engine NoOps inserted just before, one wait per NoOp. InstISA is
    skipped (fixed-length encoding)."""
    moved = 0
    for f in nc.m.functions:
        for b in f.blocks:
            insts = b.instructions
            i = 0
            while i < len(insts):
                inst = insts[i]
                if inst.sync_info is not None:
                    # ISA instructions have fixed-length encoding: they can
                    # carry no waits at all, so move every wait to NoOps
                    lim = 0 if inst.opcode == "ISA" else max_waits
                    si = inst.sync_info
                    waits = list(si.on_wait)
                    if len(waits) > lim:
                        keep = waits[len(waits) - lim :] if lim else []
                        for w in waits[: len(waits) - lim]:
                            _nop_ctr[0] += 1
                            moved += 1
                            nop = mybir.InstNoOp(
                                name=f"I-waitnop-{_nop_ctr[0]}", ins=[], outs=[]
                            )
                            nop.engine = inst.engine
                            nop.sync_info = mybir.SyncInfo(on_wait=[w], on_update=[])
                            insts.insert(i, nop)
                            i += 1
                        si.on_wait = keep
                        inst.sync_info = si
                i += 1
    return moved


def _attention_body(ctx: ExitStack, tc, inp, out_ap):
    nc = tc.nc

    persist = ctx.enter_context(tc.tile_pool(name="persist", bufs=1))
    wpool = ctx.enter_context(tc.tile_pool(name="wpool", bufs=1))
    xtp = ctx.enter_context(tc.tile_pool(name="xtp", bufs=2))
    qtmp_p = ctx.enter_context(tc.tile_pool(name="qtmp", bufs=3))
    tsh_p = ctx.enter_context(tc.tile_pool(name="tsh", bufs=3))
    tb2_p = ctx.enter_context(tc.tile_pool(name="tb2", bufs=2))
    exp_p = ctx.enter_context(tc.tile_pool(name="expp", bufs=4))
    rrec_p = ctx.enter_context(tc.tile_pool(name="rrec", bufs=2))
    vtmp_p = ctx.enter_context(tc.tile_pool(name="vtmp", bufs=2))
    tout_p = ctx.enter_context(tc.tile_pool(name="toutp", bufs=3))
    ps_pair = ctx.enter_context(tc.tile_pool(name="ps_pair", bufs=2, space="PSUM"))
    ps_uvec = ctx.enter_context(tc.tile_pool(name="ps_uvec", bufs=2, space="PSUM"))
    ps_work = ctx.enter_context(tc.tile_pool(name="ps_work", bufs=2, space="PSUM"))

    # ---- persistent tensors ----
    qrt = persist.tile([128, MT, S], BF16)      # rotated Q^T  (d-major)
    krt = persist.tile([128, MT, S], BF16)      # rotated K^T
    vext = persist.tile([128, NST, HPC * 65], BF16)  # V tiles + ones col per head
    vecT = persist.tile([128, MT, S], BF16)     # normalized attention output^T
    cos_sb = persist.tile([128, S], BF16)
    sin_sb = persist.tile([128, S], BF16)
    wo_sb = persist.tile([128, MT, D], BF16)
    mdiag_sb = persist.tile([128, 128], BF16)   # binary causal mask, diag block^T
    bq_sb = persist.tile([128, MT], F32)
    uvz = persist.tile([96, 6, 512], BF16)      # rotating evac buffers
    rtz = persist.tile([96, 4, 512], F32)       # rotating strided-recip buffers

    # weights first (per d-tile so the first matmuls start early), then consts
    wq_sb = wpool.tile([128, NDT, DC], BF16)
    wk_sb = wpool.tile([128, NDT, DC], BF16)
    wv_sb = wpool.tile([128, NDT, DC], BF16)
    xT_view = inp["xT"].rearrange("(dt p) s -> p dt s", p=128)
    xts = [
        xtp.tile([128, NDT, 512], BF16, tag="xt", name=f"xt{sb}") for sb in range(2)
    ]
    # startup order matters: everything rides the sync HWDGE ring FIFO, so
    # place each tensor just before its first consumer needs it; the rest of
    # the loads are woven between the round-0 projection pieces below
    for dt in range(NDT):
        nc.sync.dma_start(xts[0][:, dt, :], xT_view[:, dt, 0:512])
        nc.sync.dma_start(
            wq_sb[:, dt, :],
            inp["wqT"].rearrange("(dt p) o -> p dt o", p=128)[:, dt, :],
        )
    nc.sync.dma_start(bq_sb[:, :], inp["bqc"])
    nc.gpsimd.memset(uvz[64:96, :, :], 0.0)
    nc.gpsimd.memset(rtz[64:96, :, :], 0.0)
    nc.sync.dma_start(cos_sb[:, :], inp["cosT"])
    nc.sync.dma_start(sin_sb[:, :], inp["sinT"])

    def load_rest_of_consts(step):
        if step == 0:
            for dt in range(NDT):
                nc.sync.dma_start(
                    wv_sb[:, dt, :],
                    inp["wvT"].rearrange("(dt p) o -> p dt o", p=128)[:, dt, :],
                )
        elif step == 1:
            for dt in range(NDT):
                nc.sync.dma_start(
                    wk_sb[:, dt, :],
                    inp["wkT"].rearrange("(dt p) o -> p dt o", p=128)[:, dt, :],
                )
            nc.sync.dma_start(mdiag_sb[:, :], inp["mdiagT"])
        elif step == 2:
            # ones columns of vext (col 64 of each head slot, every k-tile)
            vones_dst = vext[:, :, :].rearrange("p st (h e) -> p st h e", e=65)[
                :, :, :, 64:65
            ]
            nc.sync.dma_start(
                vones_dst,
                inp["vones"].rearrange("p (st h e) -> p st h e", st=NST, h=HPC),
            )
        elif step == 3:
            for dt in range(NDT):
                nc.sync.dma_start(xts[1][:, dt, :], xT_view[:, dt, 512:1024])
            nc.sync.dma_start(
                wo_sb[:, :, :], inp["woT"].rearrange("(mt p) o -> p mt o", p=128)
            )

    def prefetch_xt(sb):
        xtn = xtp.tile([128, NDT, 512], BF16, tag="xt", name=f"xt{sb}")
        for dt in range(NDT):
            nc.sync.dma_start(
                xtn[:, dt, :], xT_view[:, dt, sb * 512 : (sb + 1) * 512]
            )
        return xtn

    # ---- phase pieces ----
    def qk_proj_piece(m, sb, which, xt):
        """Returns (a, b): a = matmuls + PSUM evac + shift DMAs, b = RoPE
        muls. Emitting b a few fill slots after a gives the shift DMAs slack
        before their consumer issues."""
        ssl = slice(sb * 512, (sb + 1) * 512)
        is_q = which == 0
        dst = qrt if is_q else krt
        w_sb = wq_sb if is_q else wk_sb
        tag_q = "q" if is_q else "k"
        state = {}

        def run_a():
            psq = ps_work.tile([128, 512], F32, tag="w", name=f"psq{tag_q}_{m}_{sb}")
            for dt in range(NDT):
                nc.tensor.matmul(
                    psq[:, :],
                    w_sb[:, dt, m * 128 : (m + 1) * 128],
                    xt[:, dt, :],
                    start=(dt == 0),
                    stop=(dt == NDT - 1),
                )
            qt = qtmp_p.tile([128, 512], BF16, tag="qt", name=f"qt{tag_q}_{m}_{sb}")
            if is_q:
                nc.scalar.activation(
                    qt[:, :], psq[:, :], AF.Identity, bias=bq_sb[:, m : m + 1]
                )
            else:
                nc.scalar.copy(qt[:, :], psq[:, :])
            # rotate_half partition shift p ^ 32 via 4 contiguous DMAs
            sh = tsh_p.tile([128, 512], BF16, tag="sh", name=f"sh{tag_q}_{m}_{sb}")
            for base in (0, 64):
                nc.sync.dma_start(
                    sh[base : base + 32, :], qt[base + 32 : base + 64, :],
                    single_packet=True,
                )
                nc.sync.dma_start(
                    sh[base + 32 : base + 64, :], qt[base : base + 32, :],
                    single_packet=True,
                )
            state["qt"], state["sh"] = qt, sh

        def run_b():
            qt, sh = state["qt"], state["sh"]
            dsl = dst[:, m, ssl]
            tb2 = tb2_p.tile([128, 512], BF16, tag="tb2", name=f"tb2{tag_q}_{m}_{sb}")
            nc.vector.tensor_mul(dsl, qt[:, :], cos_sb[:, ssl])
            nc.vector.tensor_mul(tb2[:, :], sh[:, :], sin_sb[:, ssl])
            nc.vector.tensor_add(dsl, dsl, tb2[:, :])

        return run_a, run_b

    def v_proj_piece(st, xt):
        def run():
            psv = ps_work.tile([128, 512], F32, tag="w", name=f"psv_{st}")[:, 0:256]
            for dt in range(NDT):
                nc.tensor.matmul(
                    psv[:, :],
                    xt[:, dt, (st % 4) * 128 : (st % 4 + 1) * 128],
                    wv_sb[:, dt, :],
                    start=(dt == 0),
                    stop=(dt == NDT - 1),
                )
            vdst = vext[:, st, :].rearrange("p (h e) -> p h e", e=65)[:, :, 0:64]
            nc.vector.tensor_copy(vdst, psv[:, :].rearrange("p (h e) -> p h e", e=64))
        return run

    def outproj_piece(qt_i):
        def run():
            qsl = slice(qt_i * 128, (qt_i + 1) * 128)
            to = tout_p.tile([128, 2, 512], BF16, tag="to", name=f"to_{qt_i}")
            for oc in range(2):
                osl = slice(oc * 512, (oc + 1) * 512)
                pso = ps_work.tile([128, 512], F32, tag="w", name=f"pso_{qt_i}_{oc}")
                for mt in range(MT):
                    nc.tensor.matmul(
                        pso[:, :],
                        vecT[:, mt, qsl],
                        wo_sb[:, mt, osl],
                        start=(mt == 0),
                        stop=(mt == MT - 1),
                    )
                nc.vector.tensor_copy(to[:, oc, :], pso[:, :])
            nc.scalar.dma_start(out_ap[qsl, :], to[:, :, :])
        return run

    _norm_calls = [0]

    def normalize_a(m, hp, qb, puv_t):
        """Evacuate PSUM, compute the broadcast 1/d tile. Returns (uv, rb)."""
        h = 2 * m + hp
        ci = _norm_calls[0]
        _norm_calls[0] += 1
        # evacuate PSUM immediately so the bank frees fast
        # rotating persistent slot; rows 65-95 keep their startup zeros
        # (subtile deps order the reuse after the previous reader)
        uv = uvz[:, ci % 6, :]
        nc.scalar.copy(uv[0:65, :], puv_t[:, :])
        # spread the 512 denominators over 32 partitions with the DVE
        # stream-square transpose (block t puts d[32t+p] at [p, 32t]) so the
        # reciprocal runs 16 elems/lane instead of 512 on one lane
        dct = rrec_p.tile([96, 512], BF16, tag="dt", name=f"dct_{h}_{qb}", bufs=4)
        nc.vector.transpose(dct[64:96, :], uv[64:96, :])
        dview = dct[64:96, :].rearrange("p (t j) -> p t j", j=32)[:, :, 0:1]
        # reciprocal written strided back into row-major position, second
        # transpose recovers the full 1/d row on one partition — no DRAM
        rt = rtz[:, ci % 4, :]
        rt_view = rt[64:96, :].rearrange("p (t j) -> p t j", j=32)[:, :, 0:1]
        nc.vector.reciprocal(rt_view, dview)
        rt2 = rrec_p.tile([96, 1, 512], F32, tag="r2", name=f"rt2_{h}_{qb}", bufs=4)
        nc.vector.transpose(rt2[64:96, 0, :], rt[64:96, :])
        # partition-broadcast of the 1/d row as one SBUF->SBUF DMA (stride-0
        # on a free dim); its consumer is deliberately emitted much later
        rb = rrec_p.tile([64, 512], F32, tag="rb", name=f"rb_{h}_{qb}", bufs=4)
        nc.scalar.dma_start(rb[:, :], rt2[64:65, :, :].to_broadcast([1, 64, 512]))
        return uv, rb

    def normalize_b(m, hp, qb, uv, rb):
        qsl = slice(qb * 512, (qb + 1) * 512)
        h = 2 * m + hp
        if hp == 0:
            nc.vector.tensor_mul(vecT[0:64, m, qsl], uv[0:64, :], rb[:, :])
        else:
            vt = vtmp_p.tile([64, 512], BF16, tag="vt", name=f"vt_{h}_{qb}")
            nc.vector.tensor_mul(vt[:, :], uv[0:64, :], rb[:, :])
            # cross-quadrant DVE copies (32-partition window moves) keep the
            # outproj stationary dependency on engine semaphores, not DMAs
            nc.vector.tensor_copy(vecT[64:96, m, qsl], vt[0:32, :])
            nc.vector.tensor_copy(vecT[96:128, m, qsl], vt[32:64, :])

    # ---- attention with woven fill pieces ----
    fill_queue = []
    pending_nb = []

    def emit_fill(n):
        for _ in range(n):
            if fill_queue:
                fill_queue.pop(0)()

    def attn_qb(m, qb, fill_every):
        puv = [
            ps_uvec.tile([65, 512], F32, tag="u", name=f"puv_m{m}h{hp}q{qb}")
            for hp in range(2)
        ]
        nkt = 4 * qb + 4
        pending = None
        for kt in range(nkt + 1):
            if kt < nkt:
                qb0 = kt // 4
                c0 = (kt % 4) * 128 if qb == qb0 else 0
                psc = ps_pair.tile(
                    [128, 2, 512], F32, tag="pair", name=f"psc_m{m}q{qb}k{kt}"
                )
                for hp in range(2):
                    pb = hp * 64
                    nc.tensor.matmul(
                        psc[:, hp, c0:512],
                        krt[pb : pb + 64, m, kt * 128 : (kt + 1) * 128],
                        qrt[pb : pb + 64, m, qb * 512 + c0 : (qb + 1) * 512],
                        start=True,
                        stop=True,
                    )
                et = exp_p.tile(
                    [128, 2, 512], BF16, tag="e", name=f"et_m{m}q{qb}k{kt}"
                )
                nc.scalar.activation(
                    et[:, :, c0:512], psc[:, :, c0:512], AF.Exp, scale=0.125
                )
                if qb == qb0:
                    for hp in range(2):
                        nc.gpsimd.tensor_mul(
                            et[:, hp, c0 : c0 + 128],
                            et[:, hp, c0 : c0 + 128],
                            mdiag_sb[:, :],
                        )
                cur = (kt, c0, et)
            else:
                cur = None
            if pending is not None:
                pkt, pc0, pet = pending
                for hp in range(2):
                    h = 2 * m + hp
                    nc.tensor.matmul(
                        puv[hp][:, pc0:512],
                        vext[:, pkt, h * 65 : (h + 1) * 65],
                        pet[:, hp, pc0:512],
                        start=(pkt == 0),
                        stop=(pkt == nkt - 1),
                        skip_group_check=True,
                    )
            pending = cur
            if fill_every and (kt % fill_every == fill_every - 1):
                emit_fill(1)
        for hp in range(2):
            uv, rb = normalize_a(m, hp, qb, puv[hp])
            pending_nb.append(
                lambda m=m, hp=hp, qb=qb, uv=uv, rb=rb: normalize_b(
                    m, hp, qb, uv, rb
                )
            )

    # round 0 projections standalone, const loads woven between pieces so no
    # single bulk DMA delays the round-0 shift DMAs in the ring FIFO
    qa0, qb0 = qk_proj_piece(0, 0, 0, xts[0])
    qa1, qb1 = qk_proj_piece(1, 0, 0, xts[0])
    qa0(); qa1()
    load_rest_of_consts(0)
    v_proj_piece(0, xts[0])()
    v_proj_piece(1, xts[0])()
    qb0(); qb1()
    load_rest_of_consts(1)
    ka0, kb0 = qk_proj_piece(0, 0, 1, xts[0])
    ka1, kb1 = qk_proj_piece(1, 0, 1, xts[0])
    ka0(); ka1()
    load_rest_of_consts(2)
    v_proj_piece(2, xts[0])()
    v_proj_piece(3, xts[0])()
    kb0(); kb1()
    load_rest_of_consts(3)

    for r in range(NQB):
        if r + 2 < NQB:
            xts.append(prefetch_xt(r + 2))
        # normalize division for round r-1, emitted a round late so the 1/d
        # broadcast DMAs have long since landed
        for nb_fn in pending_nb:
            nb_fn()
        pending_nb.clear()
        pieces = []
        boundary = []
        if r + 1 < NQB:
            xt_n = xts[r + 1]
            nqa0, nqb0 = qk_proj_piece(0, r + 1, 0, xt_n)
            nqa1, nqb1 = qk_proj_piece(1, r + 1, 0, xt_n)
            nka0, nkb0 = qk_proj_piece(0, r + 1, 1, xt_n)
            nka1, nkb1 = qk_proj_piece(1, r + 1, 1, xt_n)
            pieces += [
                nqa0,
                nqa1,
                v_proj_piece(4 * (r + 1) + 0, xt_n),
                v_proj_piece(4 * (r + 1) + 1, xt_n),
                nqb0,
                nqb1,
                nka0,
                nka1,
                v_proj_piece(4 * (r + 1) + 2, xt_n),
                nkb0,
                nkb1,
            ]
            # last V tile of the next round is not needed until deep into
            # round r+1 — hold it back to cover the round-boundary drain
            boundary.append(v_proj_piece(4 * (r + 1) + 3, xt_n))
        # outproj work is deferred toward the ACT-heavy late rounds: round 3
        # has the largest exp load and the least projection fill left
        if r == 1:
            boundary = [outproj_piece(0), outproj_piece(1)] + boundary
        elif r == 2:
            pieces = pieces + [outproj_piece(4), outproj_piece(5)]
            boundary = [outproj_piece(2), outproj_piece(3)] + boundary
        elif r == 3:
            pieces = pieces + [outproj_piece(i) for i in range(6, 12)]
        fill_queue.extend(pieces)
        nkts = 2 * (4 * r + 4)
        fill_every = max(1, nkts // (len(fill_queue) + 1)) if fill_queue else 0
        attn_qb(0, r, fill_every)
        attn_qb(1, r, fill_every)
        emit_fill(len(fill_queue))
        for piece in boundary:
            piece()

    for nb_fn in pending_nb:
        nb_fn()
    pending_nb.clear()
    for i in range(4):
        outproj_piece(12 + i)()


def build_bass(fix_waits=True):
    nc = bass.Bass("TRN2", debug=False)
    inp = {}

    def din(name, shape, dtype=BF16):
        inp[name] = nc.dram_tensor(name, list(shape), dtype, kind="ExternalInput").ap()

    din("xT", (D, S))
    din("wqT", (D, DC))
    din("wkT", (D, DC))
    din("wvT", (D, DC))
    din("bqc", (128, MT), F32)
    din("cosT", (128, S))
    din("sinT", (128, S))
    din("mdiagT", (128, 128))
    din("woT", (DC, D))
    din("vones", (128, NST * HPC))
    out_ap = nc.dram_tensor("out", [S, D], BF16, kind="ExternalOutput").ap()

    with tile.TileContext(nc) as tc:
        with ExitStack() as ctx:
            _attention_body(ctx, tc, inp, out_ap)
    if fix_waits:
        fix_engine_waits(nc)
    return nc


# ---- host-side sharding / prep ----


def make_core_inputs(x, mask, cos, sin, wq, bq, wk, wv, bv, wo):
    """Returns list of 8 input dicts (core c = batch c//4, head-group c%4)."""
    bf16 = ml_dtypes.bfloat16
    x = np.ascontiguousarray(x, dtype=np.float32)
    p = np.arange(128)
    pf = p % 64
    cosT = np.ascontiguousarray(cos.T[pf, :]).astype(bf16)          # [128, S]
    sgn = np.where(pf < 32, -1.0, 1.0).astype(np.float32)
    sinT = np.ascontiguousarray(sgn[:, None] * sin.T[pf, :]).astype(bf16)
    mdiagT = np.ascontiguousarray(
        (mask[0:128, 0:128].T == 0).astype(np.float32)
    ).astype(bf16)
    vones = np.ones((128, NST * HPC), dtype=bf16)

    in_maps = []
    for c in range(8):
        b, g = c // 4, c % 4
        rows = np.arange(g * DC, (g + 1) * DC)
        bqc = np.ascontiguousarray(bq[rows].reshape(MT, 128).T, dtype=np.float32)
        in_maps.append({
            "xT": np.ascontiguousarray(x[b].T).astype(bf16),
            "wqT": np.ascontiguousarray(wq[rows].T).astype(bf16),
            "wkT": np.ascontiguousarray(wk[rows].T).astype(bf16),
            "wvT": np.ascontiguousarray(wv[rows].T).astype(bf16),
            "bqc": bqc,
            "cosT": cosT,
            "sinT": sinT,
            "mdiagT": mdiagT,
            "woT": np.ascontiguousarray(wo[:, rows].T).astype(bf16),
            "vones": vones,
        })
    return in_maps


_NC_CACHE = []


def kernel(x, mask, cos, sin, wq, bq, wk, wv, bv, wo, bo):
    x = np.asarray(x, dtype=np.float32)
    in_maps = make_core_inputs(
        x, np.asarray(mask), np.asarray(cos), np.asarray(sin),
        np.asarray(wq), np.asarray(bq), np.asarray(wk), np.asarray(wv),
        np.asarray(bv), np.asarray(wo),
    )
    if not _NC_CACHE:
        _NC_CACHE.append(build_bass())
    nc = _NC_CACHE[0]
    res = run_bass_kernel_spmd(nc, in_maps, core_ids=list(range(8)))
    out = np.zeros((B, S, D), dtype=np.float32)
    for c in range(8):
        out[c // 4] += np.asarray(res.results[c]["out"], dtype=np.float32)
    # V-bias folds into a constant output row: vec_norm += bv  ->  out += bv @ Wo^T
    bvw = np.asarray(bv, dtype=np.float32) @ np.asarray(wo, dtype=np.float32).T
    out += (bvw + np.asarray(bo, dtype=np.float32))[None, None, :]
    return out


# revision 45
# speedup vs baseline: 1.1081x; 1.1081x over previous
"""Multi-head causal attention (B=2,S=2048,D=1024,H=16,RoPE) on 8 TRN2 NeuronCores.

Sharding: core c handles batch b=c//4, head-group g=c%4 (4 heads each).
Wq/Wk/Wv column-sharded per head group, Wo row-sharded; the all-reduce over
head groups is realized as a host-side partial sum at gather time.

Per-core kernel, all matmul operands bf16 (fp32 PSUM accumulation):
  Projection round r: QKV projections for s-block r from pre-transposed x,
    Q/K kept feature-major [d, s], RoPE'd via partition-shift DMAs + DVE;
    V natural [s, d] with a ones column per head (softmax denominators ride
    the AV matmul).
  Attention (m, qb): per head-pair m the two heads' score matmuls are
    row-tiled (K=64 at partition bases 0/64) into one 2-bank PSUM pair tile
    and run concurrently; a single paired exp [128, 2, 512] on ACT covers
    both heads; binary diag-mask multiply on gpsimd; AV accumulation into
    [65, 512] PSUM per head.
  Weaving: projection round r+1 and output-projection round r-1 are emitted
    as fill pieces inside attention round r's kt loop so the ACT-bound
    attention stretches keep the PE busy.
  Normalize: puv evacuated to SBUF bf16 immediately (frees the PSUM bank),
    reciprocal of the denominator row on DVE, partition-broadcast via DMA on
    the scalar HWDGE ring, divide on DVE while writing vecT.
  Output projection: vecT @ Wo per 128-q chunk, bf16 partial out -> DRAM.
"""
import numpy as np
import ml_dtypes
from contextlib import ExitStack

import concourse.bass as bass
import concourse.tile as tile
from concourse import library_config, mybir
from concourse.bass_utils import run_bass_kernel_spmd

B, S, D, H, HD = 2, 2048, 1024, 16, 64
HPC = 4            # heads per core
DC = HPC * HD      # 256 features per core
NDT = D // 128     # 8 input-dim tiles
NST = S // 128     # 16 sequence/key tiles
NQB = S // 512     # 4 query blocks
MT = DC // 128     # 2 feature m-tiles for Q/K/vec

F32 = mybir.dt.float32
BF16 = mybir.dt.bfloat16
AF = mybir.ActivationFunctionType

_nop_ctr = [0]


def fix_engine_waits(nc, max_waits=1):
    """This walrus build rejects any engine instruction with >1 sync wait
    (single wait slot per instruction struct). Move excess waits onto
    same-engine NoOps inserted just before, one wait per NoOp. InstISA is
    skipped (fixed-length encoding)."""
    moved = 0
    for f in nc.m.functions:
        for b in f.blocks:
            insts = b.instructions
            i = 0
            while i < len(insts):
                inst = insts[i]
                if inst.sync_info is not None:
                    # ISA instructions have fixed-length encoding: they can
                    # carry no waits at all, so move every wait to NoOps
                    lim = 0 if inst.opcode == "ISA" else max_waits
                    si = inst.sync_info
                    waits = list(si.on_wait)
                    if len(waits) > lim:
                        keep = waits[len(waits) - lim :] if lim else []
                        for w in waits[: len(waits) - lim]:
                            _nop_ctr[0] += 1
                            moved += 1
                            nop = mybir.InstNoOp(
                                name=f"I-waitnop-{_nop_ctr[0]}", ins=[], outs=[]
                            )
                            nop.engine = inst.engine
                            nop.sync_info = mybir.SyncInfo(on_wait=[w], on_update=[])
                            insts.insert(i, nop)
                            i += 1
                        si.on_wait = keep
                        inst.sync_info = si
                i += 1
    return moved


def _attention_body(ctx: ExitStack, tc, inp, out_ap):
    nc = tc.nc

    persist = ctx.enter_context(tc.tile_pool(name="persist", bufs=1))
    wpool = ctx.enter_context(tc.tile_pool(name="wpool", bufs=1))
    xtp = ctx.enter_context(tc.tile_pool(name="xtp", bufs=2))
    qtmp_p = ctx.enter_context(tc.tile_pool(name="qtmp", bufs=3))
    tsh_p = ctx.enter_context(tc.tile_pool(name="tsh", bufs=3))
    tb2_p = ctx.enter_context(tc.tile_pool(name="tb2", bufs=2))
    exp_p = ctx.enter_context(tc.tile_pool(name="expp", bufs=4))
    rrec_p = ctx.enter_context(tc.tile_pool(name="rrec", bufs=2))
    vtmp_p = ctx.enter_context(tc.tile_pool(name="vtmp", bufs=2))
    tout_p = ctx.enter_context(tc.tile_pool(name="toutp", bufs=2))
    ps_pair = ctx.enter_context(tc.tile_pool(name="ps_pair", bufs=2, space="PSUM"))
    ps_uvec = ctx.enter_context(tc.tile_pool(name="ps_uvec", bufs=2, space="PSUM"))
    ps_work = ctx.enter_context(tc.tile_pool(name="ps_work", bufs=2, space="PSUM"))

    # ---- persistent tensors ----
    qrt = persist.tile([128, MT, S], BF16)      # rotated Q^T  (d-major)
    krt = persist.tile([128, MT, S], BF16)      # rotated K^T
    vext = persist.tile([128, NST, HPC * 65], BF16)  # V tiles + ones col per head
    vecT = persist.tile([128, MT, S], BF16)     # normalized attention output^T
    cos_sb = persist.tile([128, S], BF16)
    sin_sb = persist.tile([128, S], BF16)
    wo_sb = persist.tile([128, MT, D], BF16)
    mdiag_sb = persist.tile([128, 128], BF16)   # binary causal mask, diag block^T
    bq_sb = persist.tile([128, MT], F32)
    uvz = persist.tile([96, 6, 512], BF16)      # rotating evac buffers
    rtz = persist.tile([96, 4, 512], F32)       # rotating strided-recip buffers

    # weights first (per d-tile so the first matmuls start early), then consts
    wq_sb = wpool.tile([128, NDT, DC], BF16)
    wk_sb = wpool.tile([128, NDT, DC], BF16)
    wv_sb = wpool.tile([128, NDT, DC], BF16)
    xT_view = inp["xT"].rearrange("(dt p) s -> p dt s", p=128)
    xts = [
        xtp.tile([128, NDT, 512], BF16, tag="xt", name=f"xt{sb}") for sb in range(2)
    ]
    # startup order matters: everything rides the sync HWDGE ring FIFO, so
    # place each tensor just before its first consumer needs it; the rest of
    # the loads are woven between the round-0 projection pieces below
    for dt in range(NDT):
        nc.sync.dma_start(xts[0][:, dt, :], xT_view[:, dt, 0:512])
        nc.sync.dma_start(
            wq_sb[:, dt, :],
            inp["wqT"].rearrange("(dt p) o -> p dt o", p=128)[:, dt, :],
        )
    nc.sync.dma_start(bq_sb[:, :], inp["bqc"])
    nc.gpsimd.memset(uvz[64:96, :, :], 0.0)
    nc.gpsimd.memset(rtz[64:96, :, :], 0.0)
    nc.sync.dma_start(cos_sb[:, :], inp["cosT"])
    nc.sync.dma_start(sin_sb[:, :], inp["sinT"])

    def load_rest_of_consts(step):
        if step == 0:
            for dt in range(NDT):
                nc.sync.dma_start(
                    wv_sb[:, dt, :],
                    inp["wvT"].rearrange("(dt p) o -> p dt o", p=128)[:, dt, :],
                )
        elif step == 1:
            for dt in range(NDT):
                nc.sync.dma_start(
                    wk_sb[:, dt, :],
                    inp["wkT"].rearrange("(dt p) o -> p dt o", p=128)[:, dt, :],
                )
            nc.sync.dma_start(mdiag_sb[:, :], inp["mdiagT"])
        elif step == 2:
            # ones columns of vext (col 64 of each head slot, every k-tile)
            vones_dst = vext[:, :, :].rearrange("p st (h e) -> p st h e", e=65)[
                :, :, :, 64:65
            ]
            nc.sync.dma_start(
                vones_dst,
                inp["vones"].rearrange("p (st h e) -> p st h e", st=NST, h=HPC),
            )
        elif step == 3:
            for dt in range(NDT):
                nc.sync.dma_start(xts[1][:, dt, :], xT_view[:, dt, 512:1024])
            nc.sync.dma_start(
                wo_sb[:, :, :], inp["woT"].rearrange("(mt p) o -> p mt o", p=128)
            )

    def prefetch_xt(sb):
        xtn = xtp.tile([128, NDT, 512], BF16, tag="xt", name=f"xt{sb}")
        for dt in range(NDT):
            nc.sync.dma_start(
                xtn[:, dt, :], xT_view[:, dt, sb * 512 : (sb + 1) * 512]
            )
        return xtn

    # ---- phase pieces ----
    def qk_proj_piece(m, sb, which, xt):
        """Returns (a, b): a = matmuls + PSUM evac + shift DMAs, b = RoPE
        muls. Emitting b a few fill slots after a gives the shift DMAs slack
        before their consumer issues."""
        ssl = slice(sb * 512, (sb + 1) * 512)
        is_q = which == 0
        dst = qrt if is_q else krt
        w_sb = wq_sb if is_q else wk_sb
        tag_q = "q" if is_q else "k"
        state = {}

        def run_a():
            psq = ps_work.tile([128, 512], F32, tag="w", name=f"psq{tag_q}_{m}_{sb}")
            for dt in range(NDT):
                nc.tensor.matmul(
                    psq[:, :],
                    w_sb[:, dt, m * 128 : (m + 1) * 128],
                    xt[:, dt, :],
                    start=(dt == 0),
                    stop=(dt == NDT - 1),
                )
            qt = qtmp_p.tile([128, 512], BF16, tag="qt", name=f"qt{tag_q}_{m}_{sb}")
            if is_q:
                nc.scalar.activation(
                    qt[:, :], psq[:, :], AF.Identity, bias=bq_sb[:, m : m + 1]
                )
            else:
                nc.scalar.copy(qt[:, :], psq[:, :])
            # rotate_half partition shift p ^ 32 via 4 contiguous DMAs
            sh = tsh_p.tile([128, 512], BF16, tag="sh", name=f"sh{tag_q}_{m}_{sb}")
            for base in (0, 64):
                nc.sync.dma_start(
                    sh[base : base + 32, :], qt[base + 32 : base + 64, :],
                    single_packet=True,
                )
                nc.sync.dma_start(
                    sh[base + 32 : base + 64, :], qt[base : base + 32, :],
                    single_packet=True,
                )
            state["qt"], state["sh"] = qt, sh

        def run_b():
            qt, sh = state["qt"], state["sh"]
            dsl = dst[:, m, ssl]
            tb2 = tb2_p.tile([128, 512], BF16, tag="tb2", name=f"tb2{tag_q}_{m}_{sb}")
            nc.vector.tensor_mul(dsl, qt[:, :], cos_sb[:, ssl])
            nc.vector.tensor_mul(tb2[:, :], sh[:, :], sin_sb[:, ssl])
            nc.vector.tensor_add(dsl, dsl, tb2[:, :])

        return run_a, run_b

    def v_proj_piece(st, xt):
        def run():
            psv = ps_work.tile([128, 512], F32, tag="w", name=f"psv_{st}")[:, 0:256]
            for dt in range(NDT):
                nc.tensor.matmul(
                    psv[:, :],
                    xt[:, dt, (st % 4) * 128 : (st % 4 + 1) * 128],
                    wv_sb[:, dt, :],
                    start=(dt == 0),
                    stop=(dt == NDT - 1),
                )
            vdst = vext[:, st, :].rearrange("p (h e) -> p h e", e=65)[:, :, 0:64]
            nc.vector.tensor_copy(vdst, psv[:, :].rearrange("p (h e) -> p h e", e=64))
        return run

    def outproj_piece(qt_i):
        def run():
            qsl = slice(qt_i * 128, (qt_i + 1) * 128)
            to = tout_p.tile([128, 2, 512], BF16, tag="to", name=f"to_{qt_i}")
            for oc in range(2):
                osl = slice(oc * 512, (oc + 1) * 512)
                pso = ps_work.tile([128, 512], F32, tag="w", name=f"pso_{qt_i}_{oc}")
                for mt in range(MT):
                    nc.tensor.matmul(
                        pso[:, :],
                        vecT[:, mt, qsl],
                        wo_sb[:, mt, osl],
                        start=(mt == 0),
                        stop=(mt == MT - 1),
                    )
                nc.vector.tensor_copy(to[:, oc, :], pso[:, :])
            nc.scalar.dma_start(out_ap[qsl, :], to[:, :, :])
        return run

    _norm_calls = [0]

    def normalize_a(m, hp, qb, puv_t):
        """Evacuate PSUM, compute the broadcast 1/d tile. Returns (uv, rb)."""
        h = 2 * m + hp
        ci = _norm_calls[0]
        _norm_calls[0] += 1
        # evacuate PSUM immediately so the bank frees fast
        # rotating persistent slot; rows 65-95 keep their startup zeros
        # (subtile deps order the reuse after the previous reader)
        uv = uvz[:, ci % 6, :]
        nc.scalar.copy(uv[0:65, :], puv_t[:, :])
        # spread the 512 denominators over 32 partitions with the DVE
        # stream-square transpose (block t puts d[32t+p] at [p, 32t]) so the
        # reciprocal runs 16 elems/lane instead of 512 on one lane
        dct = rrec_p.tile([96, 512], BF16, tag="dt", name=f"dct_{h}_{qb}", bufs=4)
        nc.vector.transpose(dct[64:96, :], uv[64:96, :])
        dview = dct[64:96, :].rearrange("p (t j) -> p t j", j=32)[:, :, 0:1]
        # reciprocal written strided back into row-major position, second
        # transpose recovers the full 1/d row on one partition — no DRAM
        rt = rtz[:, ci % 4, :]
        rt_view = rt[64:96, :].rearrange("p (t j) -> p t j", j=32)[:, :, 0:1]
        nc.vector.reciprocal(rt_view, dview)
        rt2 = rrec_p.tile([96, 1, 512], F32, tag="r2", name=f"rt2_{h}_{qb}", bufs=4)
        nc.vector.transpose(rt2[64:96, 0, :], rt[64:96, :])
        # partition-broadcast of the 1/d row as one SBUF->SBUF DMA (stride-0
        # on a free dim); its consumer is deliberately emitted much later
        rb = rrec_p.tile([64, 512], F32, tag="rb", name=f"rb_{h}_{qb}", bufs=4)
        nc.scalar.dma_start(rb[:, :], rt2[64:65, :, :].to_broadcast([1, 64, 512]))
        return uv, rb

    def normalize_b(m, hp, qb, uv, rb):
        qsl = slice(qb * 512, (qb + 1) * 512)
        h = 2 * m + hp
        if hp == 0:
            nc.vector.tensor_mul(vecT[0:64, m, qsl], uv[0:64, :], rb[:, :])
        else:
            vt = vtmp_p.tile([64, 512], BF16, tag="vt", name=f"vt_{h}_{qb}")
            nc.vector.tensor_mul(vt[:, :], uv[0:64, :], rb[:, :])
            # cross-quadrant DVE copies (32-partition window moves) keep the
            # outproj stationary dependency on engine semaphores, not DMAs
            nc.vector.tensor_copy(vecT[64:96, m, qsl], vt[0:32, :])
            nc.vector.tensor_copy(vecT[96:128, m, qsl], vt[32:64, :])

    # ---- attention with woven fill pieces ----
    fill_queue = []
    pending_nb = []

    def emit_fill(n):
        for _ in range(n):
            if fill_queue:
                fill_queue.pop(0)()

    def attn_qb(m, qb, fill_every):
        puv = [
            ps_uvec.tile([65, 512], F32, tag="u", name=f"puv_m{m}h{hp}q{qb}")
            for hp in range(2)
        ]
        nkt = 4 * qb + 4
        pending = None
        for kt in range(nkt + 1):
            if kt < nkt:
                qb0 = kt // 4
                c0 = (kt % 4) * 128 if qb == qb0 else 0
                psc = ps_pair.tile(
                    [128, 2, 512], F32, tag="pair", name=f"psc_m{m}q{qb}k{kt}"
                )
                for hp in range(2):
                    pb = hp * 64
                    nc.tensor.matmul(
                        psc[:, hp, c0:512],
                        krt[pb : pb + 64, m, kt * 128 : (kt + 1) * 128],
                        qrt[pb : pb + 64, m, qb * 512 + c0 : (qb + 1) * 512],
                        start=True,
                        stop=True,
                    )
                et = exp_p.tile(
                    [128, 2, 512], BF16, tag="e", name=f"et_m{m}q{qb}k{kt}"
                )
                nc.scalar.activation(
                    et[:, :, c0:512], psc[:, :, c0:512], AF.Exp, scale=0.125
                )
                if qb == qb0:
                    for hp in range(2):
                        nc.gpsimd.tensor_mul(
                            et[:, hp, c0 : c0 + 128],
                            et[:, hp, c0 : c0 + 128],
                            mdiag_sb[:, :],
                        )
                cur = (kt, c0, et)
            else:
                cur = None
            if pending is not None:
                pkt, pc0, pet = pending
                for hp in range(2):
                    h = 2 * m + hp
                    nc.tensor.matmul(
                        puv[hp][:, pc0:512],
                        vext[:, pkt, h * 65 : (h + 1) * 65],
                        pet[:, hp, pc0:512],
                        start=(pkt == 0),
                        stop=(pkt == nkt - 1),
                        skip_group_check=True,
                    )
            pending = cur
            if fill_every and (kt % fill_every == fill_every - 1):
                emit_fill(1)
        for hp in range(2):
            uv, rb = normalize_a(m, hp, qb, puv[hp])
            pending_nb.append(
                lambda m=m, hp=hp, qb=qb, uv=uv, rb=rb: normalize_b(
                    m, hp, qb, uv, rb
                )
            )

    # round 0 projections standalone, const loads woven between pieces so no
    # single bulk DMA delays the round-0 shift DMAs in the ring FIFO
    qa0, qb0 = qk_proj_piece(0, 0, 0, xts[0])
    qa1, qb1 = qk_proj_piece(1, 0, 0, xts[0])
    qa0(); qa1()
    load_rest_of_consts(0)
    v_proj_piece(0, xts[0])()
    v_proj_piece(1, xts[0])()
    qb0(); qb1()
    load_rest_of_consts(1)
    ka0, kb0 = qk_proj_piece(0, 0, 1, xts[0])
    ka1, kb1 = qk_proj_piece(1, 0, 1, xts[0])
    ka0(); ka1()
    load_rest_of_consts(2)
    v_proj_piece(2, xts[0])()
    v_proj_piece(3, xts[0])()
    kb0(); kb1()
    load_rest_of_consts(3)

    for r in range(NQB):
        if r + 2 < NQB:
            xts.append(prefetch_xt(r + 2))
        # normalize division for round r-1, emitted a round late so the 1/d
        # broadcast DMAs have long since landed
        for nb_fn in pending_nb:
            nb_fn()
        pending_nb.clear()
        pieces = []
        boundary = []
        if r + 1 < NQB:
            xt_n = xts[r + 1]
            nqa0, nqb0 = qk_proj_piece(0, r + 1, 0, xt_n)
            nqa1, nqb1 = qk_proj_piece(1, r + 1, 0, xt_n)
            nka0, nkb0 = qk_proj_piece(0, r + 1, 1, xt_n)
            nka1, nkb1 = qk_proj_piece(1, r + 1, 1, xt_n)
            pieces += [
                nqa0,
                nqa1,
                v_proj_piece(4 * (r + 1) + 0, xt_n),
                v_proj_piece(4 * (r + 1) + 1, xt_n),
                nqb0,
                nqb1,
                nka0,
                nka1,
                v_proj_piece(4 * (r + 1) + 2, xt_n),
                nkb0,
                nkb1,
            ]
            # last V tile of the next round is not needed until deep into
            # round r+1 — hold it back to cover the round-boundary drain
            boundary.append(v_proj_piece(4 * (r + 1) + 3, xt_n))
        # outproj work is deferred toward the ACT-heavy late rounds: round 3
        # has the largest exp load and the least projection fill left
        if r == 1:
            boundary = [outproj_piece(0), outproj_piece(1)] + boundary
        elif r == 2:
            pieces = pieces + [outproj_piece(4), outproj_piece(5)]
            boundary = [outproj_piece(2), outproj_piece(3)] + boundary
        elif r == 3:
            pieces = pieces + [outproj_piece(i) for i in range(6, 12)]
        fill_queue.extend(pieces)
        nkts = 2 * (4 * r + 4)
        fill_every = max(1, nkts // (len(fill_queue) + 1)) if fill_queue else 0
        attn_qb(0, r, fill_every)
        attn_qb(1, r, fill_every)
        emit_fill(len(fill_queue))
        for piece in boundary:
            piece()

    for nb_fn in pending_nb:
        nb_fn()
    pending_nb.clear()
    for i in range(4):
        outproj_piece(12 + i)()


def build_bass(fix_waits=True):
    nc = bass.Bass("TRN2", debug=False)
    inp = {}

    def din(name, shape, dtype=BF16):
        inp[name] = nc.dram_tensor(name, list(shape), dtype, kind="ExternalInput").ap()

    din("xT", (D, S))
    din("wqT", (D, DC))
    din("wkT", (D, DC))
    din("wvT", (D, DC))
    din("bqc", (128, MT), F32)
    din("cosT", (128, S))
    din("sinT", (128, S))
    din("mdiagT", (128, 128))
    din("woT", (DC, D))
    din("vones", (128, NST * HPC))
    out_ap = nc.dram_tensor("out", [S, D], BF16, kind="ExternalOutput").ap()

    with tile.TileContext(nc) as tc:
        with ExitStack() as ctx:
            _attention_body(ctx, tc, inp, out_ap)
    if fix_waits:
        fix_engine_waits(nc)
    return nc


# ---- host-side sharding / prep ----


def make_core_inputs(x, mask, cos, sin, wq, bq, wk, wv, bv, wo):
    """Returns list of 8 input dicts (core c = batch c//4, head-group c%4)."""
    bf16 = ml_dtypes.bfloat16
    x = np.ascontiguousarray(x, dtype=np.float32)
    p = np.arange(128)
    pf = p % 64
    cosT = np.ascontiguousarray(cos.T[pf, :]).astype(bf16)          # [128, S]
    sgn = np.where(pf < 32, -1.0, 1.0).astype(np.float32)
    sinT = np.ascontiguousarray(sgn[:, None] * sin.T[pf, :]).astype(bf16)
    mdiagT = np.ascontiguousarray(
        (mask[0:128, 0:128].T == 0).astype(np.float32)
    ).astype(bf16)
    vones = np.ones((128, NST * HPC), dtype=bf16)

    in_maps = []
    for c in range(8):
        b, g = c // 4, c % 4
        rows = np.arange(g * DC, (g + 1) * DC)
        bqc = np.ascontiguousarray(bq[rows].reshape(MT, 128).T, dtype=np.float32)
        in_maps.append({
            "xT": np.ascontiguousarray(x[b].T).astype(bf16),
            "wqT": np.ascontiguousarray(wq[rows].T).astype(bf16),
            "wkT": np.ascontiguousarray(wk[rows].T).astype(bf16),
            "wvT": np.ascontiguousarray(wv[rows].T).astype(bf16),
            "bqc": bqc,
            "cosT": cosT,
            "sinT": sinT,
            "mdiagT": mdiagT,
            "woT": np.ascontiguousarray(wo[:, rows].T).astype(bf16),
            "vones": vones,
        })
    return in_maps


_NC_CACHE = []


def kernel(x, mask, cos, sin, wq, bq, wk, wv, bv, wo, bo):
    x = np.asarray(x, dtype=np.float32)
    in_maps = make_core_inputs(
        x, np.asarray(mask), np.asarray(cos), np.asarray(sin),
        np.asarray(wq), np.asarray(bq), np.asarray(wk), np.asarray(wv),
        np.asarray(bv), np.asarray(wo),
    )
    if not _NC_CACHE:
        _NC_CACHE.append(build_bass())
    nc = _NC_CACHE[0]
    res = run_bass_kernel_spmd(nc, in_maps, core_ids=list(range(8)))
    out = np.zeros((B, S, D), dtype=np.float32)
    for c in range(8):
        out[c // 4] += np.asarray(res.results[c]["out"], dtype=np.float32)
    # V-bias folds into a constant output row: vec_norm += bv  ->  out += bv @ Wo^T
    bvw = np.asarray(bv, dtype=np.float32) @ np.asarray(wo, dtype=np.float32).T
    out += (bvw + np.asarray(bo, dtype=np.float32))[None, None, :]
    return out


# revision 46
# speedup vs baseline: 1.1144x; 1.0057x over previous
"""Multi-head causal attention (B=2,S=2048,D=1024,H=16,RoPE) on 8 TRN2 NeuronCores.

Sharding: core c handles batch b=c//4, head-group g=c%4 (4 heads each).
Wq/Wk/Wv column-sharded per head group, Wo row-sharded; the all-reduce over
head groups is realized as a host-side partial sum at gather time.

Per-core kernel, all matmul operands bf16 (fp32 PSUM accumulation):
  Projection round r: QKV projections for s-block r from pre-transposed x,
    Q/K kept feature-major [d, s], RoPE'd via partition-shift DMAs + DVE
    (pieces split in two so shift DMAs get slack before their consumers);
    V natural [s, d] with a ones column per head (softmax denominators ride
    the AV matmul; the V bias folds into a constant output row added on the
    host: out += bv @ Wo^T).
  Attention (m, qb): per head-pair m the two heads' score matmuls are
    row-tiled (K=64 at partition bases 0/64) into one 2-bank PSUM pair tile
    and run concurrently; a single paired exp [128, 2, 512-c0] on ACT covers
    both heads; binary diag-mask multiply on gpsimd; AV accumulation into
    [65, 512] PSUM per head.
  Weaving: projection round r+1 is emitted as fill pieces inside attention
    round r's kt loop, and output projections are deferred toward the
    ACT-heavy late rounds, so exp-bound attention stretches keep the PE
    busy; reserved boundary pieces cover round-transition drains.
  Normalize (race-hardened, engine-semaphore synced): puv evacuated by ACT
    into a rotating persistent SBUF buffer (frees the PSUM bank fast), DVE
    stream-transpose spreads the denominator row over 32 lanes, reciprocal,
    second transpose recovers the 1/d row, one SBUF->SBUF broadcast DMA
    (stride-0 free dim), and the divide (normalize_b) is deliberately
    emitted a round later so the broadcast has landed long before its
    consumer issues; hp1 halves reach partitions 64-127 via cross-quadrant
    32-partition DVE copies instead of DMAs.
  Output projection: vecT @ Wo per 128-q chunk, bf16 partial out -> DRAM.
  DMA rings: bulk loads + shifts on the sync HWDGE ring (fine granularity so
    no transfer blocks a latency chain), out stores + 1/d broadcasts on the
    scalar ring; the ACT/Pool queues stay compute-only.
"""
import numpy as np
import ml_dtypes
from contextlib import ExitStack

import concourse.bass as bass
import concourse.tile as tile
from concourse import library_config, mybir
from concourse.bass_utils import run_bass_kernel_spmd

B, S, D, H, HD = 2, 2048, 1024, 16, 64
HPC = 4            # heads per core
DC = HPC * HD      # 256 features per core
NDT = D // 128     # 8 input-dim tiles
NST = S // 128     # 16 sequence/key tiles
NQB = S // 512     # 4 query blocks
MT = DC // 128     # 2 feature m-tiles for Q/K/vec

F32 = mybir.dt.float32
BF16 = mybir.dt.bfloat16
AF = mybir.ActivationFunctionType

_nop_ctr = [0]


def fix_engine_waits(nc, max_waits=1):
    """This walrus build rejects any engine instruction with >1 sync wait
    (single wait slot per instruction struct). Move excess waits onto
    same-engine NoOps inserted just before, one wait per NoOp. InstISA is
    skipped (fixed-length encoding)."""
    moved = 0
    for f in nc.m.functions:
        for b in f.blocks:
            insts = b.instructions
            i = 0
            while i < len(insts):
                inst = insts[i]
                if inst.sync_info is not None:
                    # ISA instructions have fixed-length encoding: they can
                    # carry no waits at all, so move every wait to NoOps
                    lim = 0 if inst.opcode == "ISA" else max_waits
                    si = inst.sync_info
                    waits = list(si.on_wait)
                    if len(waits) > lim:
                        keep = waits[len(waits) - lim :] if lim else []
                        for w in waits[: len(waits) - lim]:
                            _nop_ctr[0] += 1
                            moved += 1
                            nop = mybir.InstNoOp(
                                name=f"I-waitnop-{_nop_ctr[0]}", ins=[], outs=[]
                            )
                            nop.engine = inst.engine
                            nop.sync_info = mybir.SyncInfo(on_wait=[w], on_update=[])
                            insts.insert(i, nop)
                            i += 1
                        si.on_wait = keep
                        inst.sync_info = si
                i += 1
    return moved


def _attention_body(ctx: ExitStack, tc, inp, out_ap):
    nc = tc.nc

    persist = ctx.enter_context(tc.tile_pool(name="persist", bufs=1))
    wpool = ctx.enter_context(tc.tile_pool(name="wpool", bufs=1))
    xtp = ctx.enter_context(tc.tile_pool(name="xtp", bufs=2))
    qtmp_p = ctx.enter_context(tc.tile_pool(name="qtmp", bufs=3))
    tsh_p = ctx.enter_context(tc.tile_pool(name="tsh", bufs=3))
    tb2_p = ctx.enter_context(tc.tile_pool(name="tb2", bufs=2))
    exp_p = ctx.enter_context(tc.tile_pool(name="expp", bufs=4))
    rrec_p = ctx.enter_context(tc.tile_pool(name="rrec", bufs=2))
    vtmp_p = ctx.enter_context(tc.tile_pool(name="vtmp", bufs=2))
    tout_p = ctx.enter_context(tc.tile_pool(name="toutp", bufs=2))
    ps_pair = ctx.enter_context(tc.tile_pool(name="ps_pair", bufs=2, space="PSUM"))
    ps_uvec = ctx.enter_context(tc.tile_pool(name="ps_uvec", bufs=2, space="PSUM"))
    ps_work = ctx.enter_context(tc.tile_pool(name="ps_work", bufs=2, space="PSUM"))

    # ---- persistent tensors ----
    qrt = persist.tile([128, MT, S], BF16)      # rotated Q^T  (d-major)
    krt = persist.tile([128, MT, S], BF16)      # rotated K^T
    vext = persist.tile([128, NST, HPC * 65], BF16)  # V tiles + ones col per head
    vecT = persist.tile([128, MT, S], BF16)     # normalized attention output^T
    cos_sb = persist.tile([128, S], BF16)
    sin_sb = persist.tile([128, S], BF16)
    wo_sb = persist.tile([128, MT, D], BF16)
    mdiag_sb = persist.tile([128, 128], BF16)   # binary causal mask, diag block^T
    bq_sb = persist.tile([128, MT], F32)
    uvz = persist.tile([96, 6, 512], BF16)      # rotating evac buffers
    rtz = persist.tile([96, 4, 512], F32)       # rotating strided-recip buffers

    # weights first (per d-tile so the first matmuls start early), then consts
    wq_sb = wpool.tile([128, NDT, DC], BF16)
    wk_sb = wpool.tile([128, NDT, DC], BF16)
    wv_sb = wpool.tile([128, NDT, DC], BF16)
    xT_view = inp["xT"].rearrange("(dt p) s -> p dt s", p=128)
    xts = [
        xtp.tile([128, NDT, 512], BF16, tag="xt", name=f"xt{sb}") for sb in range(2)
    ]
    # startup order matters: everything rides the sync HWDGE ring FIFO, so
    # place each tensor just before its first consumer needs it; the rest of
    # the loads are woven between the round-0 projection pieces below
    for dt in range(NDT):
        nc.sync.dma_start(xts[0][:, dt, :], xT_view[:, dt, 0:512])
        nc.sync.dma_start(
            wq_sb[:, dt, :],
            inp["wqT"].rearrange("(dt p) o -> p dt o", p=128)[:, dt, :],
        )
    nc.sync.dma_start(bq_sb[:, :], inp["bqc"])
    nc.gpsimd.memset(uvz[64:96, :, :], 0.0)
    nc.gpsimd.memset(rtz[64:96, :, :], 0.0)
    nc.sync.dma_start(cos_sb[:, :], inp["cosT"])
    nc.sync.dma_start(sin_sb[:, :], inp["sinT"])

    def load_rest_of_consts(step):
        if step == 0:
            for dt in range(NDT):
                nc.sync.dma_start(
                    wv_sb[:, dt, :],
                    inp["wvT"].rearrange("(dt p) o -> p dt o", p=128)[:, dt, :],
                )
        elif step == 1:
            for dt in range(NDT):
                nc.sync.dma_start(
                    wk_sb[:, dt, :],
                    inp["wkT"].rearrange("(dt p) o -> p dt o", p=128)[:, dt, :],
                )
            nc.sync.dma_start(mdiag_sb[:, :], inp["mdiagT"])
        elif step == 2:
            # ones columns of vext (col 64 of each head slot, every k-tile)
            vones_dst = vext[:, :, :].rearrange("p st (h e) -> p st h e", e=65)[
                :, :, :, 64:65
            ]
            nc.sync.dma_start(
                vones_dst,
                inp["vones"].rearrange("p (st h e) -> p st h e", st=NST, h=HPC),
            )
        elif step == 3:
            for dt in range(NDT):
                nc.sync.dma_start(xts[1][:, dt, :], xT_view[:, dt, 512:1024])
            nc.sync.dma_start(
                wo_sb[:, :, :], inp["woT"].rearrange("(mt p) o -> p mt o", p=128)
            )

    def prefetch_xt(sb):
        xtn = xtp.tile([128, NDT, 512], BF16, tag="xt", name=f"xt{sb}")
        for dt in range(NDT):
            nc.sync.dma_start(
                xtn[:, dt, :], xT_view[:, dt, sb * 512 : (sb + 1) * 512]
            )
        return xtn

    # ---- phase pieces ----
    def qk_proj_piece(m, sb, which, xt):
        """Returns (a, b): a = matmuls + PSUM evac + shift DMAs, b = RoPE
        muls. Emitting b a few fill slots after a gives the shift DMAs slack
        before their consumer issues."""
        ssl = slice(sb * 512, (sb + 1) * 512)
        is_q = which == 0
        dst = qrt if is_q else krt
        w_sb = wq_sb if is_q else wk_sb
        tag_q = "q" if is_q else "k"
        state = {}

        def run_a():
            psq = ps_work.tile([128, 512], F32, tag="w", name=f"psq{tag_q}_{m}_{sb}")
            for dt in range(NDT):
                nc.tensor.matmul(
                    psq[:, :],
                    w_sb[:, dt, m * 128 : (m + 1) * 128],
                    xt[:, dt, :],
                    start=(dt == 0),
                    stop=(dt == NDT - 1),
                )
            qt = qtmp_p.tile([128, 512], BF16, tag="qt", name=f"qt{tag_q}_{m}_{sb}")
            if is_q:
                nc.scalar.activation(
                    qt[:, :], psq[:, :], AF.Identity, bias=bq_sb[:, m : m + 1]
                )
            else:
                nc.scalar.copy(qt[:, :], psq[:, :])
            # rotate_half partition shift p ^ 32 via 4 contiguous DMAs
            sh = tsh_p.tile([128, 512], BF16, tag="sh", name=f"sh{tag_q}_{m}_{sb}")
            for base in (0, 64):
                nc.sync.dma_start(
                    sh[base : base + 32, :], qt[base + 32 : base + 64, :],
                    single_packet=True,
                )
                nc.sync.dma_start(
                    sh[base + 32 : base + 64, :], qt[base : base + 32, :],
                    single_packet=True,
                )
            state["qt"], state["sh"] = qt, sh

        def run_b():
            qt, sh = state["qt"], state["sh"]
            dsl = dst[:, m, ssl]
            tb2 = tb2_p.tile([128, 512], BF16, tag="tb2", name=f"tb2{tag_q}_{m}_{sb}")
            nc.vector.tensor_mul(dsl, qt[:, :], cos_sb[:, ssl])
            nc.vector.tensor_mul(tb2[:, :], sh[:, :], sin_sb[:, ssl])
            nc.vector.tensor_add(dsl, dsl, tb2[:, :])

        return run_a, run_b

    def v_proj_piece(st, xt):
        def run():
            psv = ps_work.tile([128, 512], F32, tag="w", name=f"psv_{st}")[:, 0:256]
            for dt in range(NDT):
                nc.tensor.matmul(
                    psv[:, :],
                    xt[:, dt, (st % 4) * 128 : (st % 4 + 1) * 128],
                    wv_sb[:, dt, :],
                    start=(dt == 0),
                    stop=(dt == NDT - 1),
                )
            vdst = vext[:, st, :].rearrange("p (h e) -> p h e", e=65)[:, :, 0:64]
            nc.vector.tensor_copy(vdst, psv[:, :].rearrange("p (h e) -> p h e", e=64))
        return run

    def outproj_piece(qt_i):
        def run():
            qsl = slice(qt_i * 128, (qt_i + 1) * 128)
            to = tout_p.tile([128, 2, 512], BF16, tag="to", name=f"to_{qt_i}")
            for oc in range(2):
                osl = slice(oc * 512, (oc + 1) * 512)
                pso = ps_work.tile([128, 512], F32, tag="w", name=f"pso_{qt_i}_{oc}")
                for mt in range(MT):
                    nc.tensor.matmul(
                        pso[:, :],
                        vecT[:, mt, qsl],
                        wo_sb[:, mt, osl],
                        start=(mt == 0),
                        stop=(mt == MT - 1),
                    )
                nc.vector.tensor_copy(to[:, oc, :], pso[:, :])
            nc.scalar.dma_start(out_ap[qsl, :], to[:, :, :])
        return run

    _norm_calls = [0]

    def normalize_a(m, hp, qb, puv_t):
        """Evacuate PSUM, compute the broadcast 1/d tile. Returns (uv, rb)."""
        h = 2 * m + hp
        ci = _norm_calls[0]
        _norm_calls[0] += 1
        # evacuate PSUM immediately so the bank frees fast
        # rotating persistent slot; rows 65-95 keep their startup zeros
        # (subtile deps order the reuse after the previous reader)
        uv = uvz[:, ci % 6, :]
        nc.scalar.copy(uv[0:65, :], puv_t[:, :])
        # spread the 512 denominators over 32 partitions with the DVE
        # stream-square transpose (block t puts d[32t+p] at [p, 32t]) so the
        # reciprocal runs 16 elems/lane instead of 512 on one lane
        dct = rrec_p.tile([96, 512], BF16, tag="dt", name=f"dct_{h}_{qb}", bufs=4)
        nc.vector.transpose(dct[64:96, :], uv[64:96, :])
        dview = dct[64:96, :].rearrange("p (t j) -> p t j", j=32)[:, :, 0:1]
        # reciprocal written strided back into row-major position, second
        # transpose recovers the full 1/d row on one partition — no DRAM
        rt = rtz[:, ci % 4, :]
        rt_view = rt[64:96, :].rearrange("p (t j) -> p t j", j=32)[:, :, 0:1]
        nc.vector.reciprocal(rt_view, dview)
        rt2 = rrec_p.tile([96, 1, 512], F32, tag="r2", name=f"rt2_{h}_{qb}", bufs=4)
        nc.vector.transpose(rt2[64:96, 0, :], rt[64:96, :])
        # partition-broadcast of the 1/d row as one SBUF->SBUF DMA (stride-0
        # on a free dim); its consumer is deliberately emitted much later
        rb = rrec_p.tile([64, 512], F32, tag="rb", name=f"rb_{h}_{qb}", bufs=4)
        nc.scalar.dma_start(rb[:, :], rt2[64:65, :, :].to_broadcast([1, 64, 512]))
        return uv, rb

    def normalize_b(m, hp, qb, uv, rb):
        qsl = slice(qb * 512, (qb + 1) * 512)
        h = 2 * m + hp
        if hp == 0:
            nc.vector.tensor_mul(vecT[0:64, m, qsl], uv[0:64, :], rb[:, :])
        else:
            vt = vtmp_p.tile([64, 512], BF16, tag="vt", name=f"vt_{h}_{qb}")
            nc.vector.tensor_mul(vt[:, :], uv[0:64, :], rb[:, :])
            # cross-quadrant DVE copies (32-partition window moves) keep the
            # outproj stationary dependency on engine semaphores, not DMAs
            nc.vector.tensor_copy(vecT[64:96, m, qsl], vt[0:32, :])
            nc.vector.tensor_copy(vecT[96:128, m, qsl], vt[32:64, :])

    # ---- attention with woven fill pieces ----
    fill_queue = []
    pending_nb = []

    def emit_fill(n):
        for _ in range(n):
            if fill_queue:
                fill_queue.pop(0)()

    def attn_qb(m, qb, fill_every):
        puv = [
            ps_uvec.tile([65, 512], F32, tag="u", name=f"puv_m{m}h{hp}q{qb}")
            for hp in range(2)
        ]
        nkt = 4 * qb + 4
        pending = None
        for kt in range(nkt + 1):
            if kt < nkt:
                qb0 = kt // 4
                c0 = (kt % 4) * 128 if qb == qb0 else 0
                psc = ps_pair.tile(
                    [128, 2, 512], F32, tag="pair", name=f"psc_m{m}q{qb}k{kt}"
                )
                for hp in range(2):
                    pb = hp * 64
                    nc.tensor.matmul(
                        psc[:, hp, c0:512],
                        krt[pb : pb + 64, m, kt * 128 : (kt + 1) * 128],
                        qrt[pb : pb + 64, m, qb * 512 + c0 : (qb + 1) * 512],
                        start=True,
                        stop=True,
                    )
                et = exp_p.tile(
                    [128, 2, 512], BF16, tag="e", name=f"et_m{m}q{qb}k{kt}"
                )
                nc.scalar.activation(
                    et[:, :, c0:512], psc[:, :, c0:512], AF.Exp, scale=0.125
                )
                if qb == qb0:
                    for hp in range(2):
                        nc.gpsimd.tensor_mul(
                            et[:, hp, c0 : c0 + 128],
                            et[:, hp, c0 : c0 + 128],
                            mdiag_sb[:, :],
                        )
                cur = (kt, c0, et)
            else:
                cur = None
            if pending is not None:
                pkt, pc0, pet = pending
                for hp in range(2):
                    h = 2 * m + hp
                    nc.tensor.matmul(
                        puv[hp][:, pc0:512],
                        vext[:, pkt, h * 65 : (h + 1) * 65],
                        pet[:, hp, pc0:512],
                        start=(pkt == 0),
                        stop=(pkt == nkt - 1),
                        skip_group_check=True,
                    )
            pending = cur
            if fill_every and (kt % fill_every == fill_every - 1):
                emit_fill(1)
        for hp in range(2):
            uv, rb = normalize_a(m, hp, qb, puv[hp])
            pending_nb.append(
                lambda m=m, hp=hp, qb=qb, uv=uv, rb=rb: normalize_b(
                    m, hp, qb, uv, rb
                )
            )

    # round 0 projections standalone, const loads woven between pieces so no
    # single bulk DMA delays the round-0 shift DMAs in the ring FIFO
    qa0, qb0 = qk_proj_piece(0, 0, 0, xts[0])
    qa1, qb1 = qk_proj_piece(1, 0, 0, xts[0])
    qa0(); qa1()
    load_rest_of_consts(0)
    v_proj_piece(0, xts[0])()
    v_proj_piece(1, xts[0])()
    qb0(); qb1()
    load_rest_of_consts(1)
    ka0, kb0 = qk_proj_piece(0, 0, 1, xts[0])
    ka1, kb1 = qk_proj_piece(1, 0, 1, xts[0])
    ka0(); ka1()
    load_rest_of_consts(2)
    v_proj_piece(2, xts[0])()
    v_proj_piece(3, xts[0])()
    kb0(); kb1()
    load_rest_of_consts(3)

    for r in range(NQB):
        if r + 2 < NQB:
            xts.append(prefetch_xt(r + 2))
        # normalize division for round r-1, emitted a round late so the 1/d
        # broadcast DMAs have long since landed
        for nb_fn in pending_nb:
            nb_fn()
        pending_nb.clear()
        pieces = []
        boundary = []
        if r + 1 < NQB:
            xt_n = xts[r + 1]
            nqa0, nqb0 = qk_proj_piece(0, r + 1, 0, xt_n)
            nqa1, nqb1 = qk_proj_piece(1, r + 1, 0, xt_n)
            nka0, nkb0 = qk_proj_piece(0, r + 1, 1, xt_n)
            nka1, nkb1 = qk_proj_piece(1, r + 1, 1, xt_n)
            pieces += [
                nqa0,
                nqa1,
                v_proj_piece(4 * (r + 1) + 0, xt_n),
                v_proj_piece(4 * (r + 1) + 1, xt_n),
                nqb0,
                nqb1,
                nka0,
                nka1,
                v_proj_piece(4 * (r + 1) + 2, xt_n),
                nkb0,
                nkb1,
            ]
            # last V tile of the next round is not needed until deep into
            # round r+1 — hold it back to cover the round-boundary drain
            boundary.append(v_proj_piece(4 * (r + 1) + 3, xt_n))
        # outproj work is deferred toward the ACT-heavy late rounds: round 3
        # has the largest exp load and the least projection fill left
        if r == 1:
            boundary = [outproj_piece(0), outproj_piece(1)] + boundary
        elif r == 2:
            pieces = pieces + [outproj_piece(4), outproj_piece(5)]
            boundary = [outproj_piece(2), outproj_piece(3)] + boundary
        elif r == 3:
            pieces = pieces + [outproj_piece(i) for i in range(6, 12)]
        fill_queue.extend(pieces)
        nkts = 2 * (4 * r + 4)
        fill_every = max(1, nkts // (len(fill_queue) + 1)) if fill_queue else 0
        attn_qb(0, r, fill_every)
        attn_qb(1, r, fill_every)
        emit_fill(len(fill_queue))
        for piece in boundary:
            piece()

    for nb_fn in pending_nb:
        nb_fn()
    pending_nb.clear()
    for i in range(4):
        outproj_piece(12 + i)()


def build_bass(fix_waits=True):
    nc = bass.Bass("TRN2", debug=False)
    inp = {}

    def din(name, shape, dtype=BF16):
        inp[name] = nc.dram_tensor(name, list(shape), dtype, kind="ExternalInput").ap()

    din("xT", (D, S))
    din("wqT", (D, DC))
    din("wkT", (D, DC))
    din("wvT", (D, DC))
    din("bqc", (128, MT), F32)
    din("cosT", (128, S))
    din("sinT", (128, S))
    din("mdiagT", (128, 128))
    din("woT", (DC, D))
    din("vones", (128, NST * HPC))
    out_ap = nc.dram_tensor("out", [S, D], BF16, kind="ExternalOutput").ap()

    with tile.TileContext(nc) as tc:
        with ExitStack() as ctx:
            _attention_body(ctx, tc, inp, out_ap)
    if fix_waits:
        fix_engine_waits(nc)
    return nc


# ---- host-side sharding / prep ----


def make_core_inputs(x, mask, cos, sin, wq, bq, wk, wv, bv, wo):
    """Returns list of 8 input dicts (core c = batch c//4, head-group c%4)."""
    bf16 = ml_dtypes.bfloat16
    x = np.ascontiguousarray(x, dtype=np.float32)
    p = np.arange(128)
    pf = p % 64
    cosT = np.ascontiguousarray(cos.T[pf, :]).astype(bf16)          # [128, S]
    sgn = np.where(pf < 32, -1.0, 1.0).astype(np.float32)
    sinT = np.ascontiguousarray(sgn[:, None] * sin.T[pf, :]).astype(bf16)
    mdiagT = np.ascontiguousarray(
        (mask[0:128, 0:128].T == 0).astype(np.float32)
    ).astype(bf16)
    vones = np.ones((128, NST * HPC), dtype=bf16)

    in_maps = []
    for c in range(8):
        b, g = c // 4, c % 4
        rows = np.arange(g * DC, (g + 1) * DC)
        bqc = np.ascontiguousarray(bq[rows].reshape(MT, 128).T, dtype=np.float32)
        in_maps.append({
            "xT": np.ascontiguousarray(x[b].T).astype(bf16),
            "wqT": np.ascontiguousarray(wq[rows].T).astype(bf16),
            "wkT": np.ascontiguousarray(wk[rows].T).astype(bf16),
            "wvT": np.ascontiguousarray(wv[rows].T).astype(bf16),
            "bqc": bqc,
            "cosT": cosT,
            "sinT": sinT,
            "mdiagT": mdiagT,
            "woT": np.ascontiguousarray(wo[:, rows].T).astype(bf16),
            "vones": vones,
        })
    return in_maps


_NC_CACHE = []


def kernel(x, mask, cos, sin, wq, bq, wk, wv, bv, wo, bo):
    x = np.asarray(x, dtype=np.float32)
    in_maps = make_core_inputs(
        x, np.asarray(mask), np.asarray(cos), np.asarray(sin),
        np.asarray(wq), np.asarray(bq), np.asarray(wk), np.asarray(wv),
        np.asarray(bv), np.asarray(wo),
    )
    if not _NC_CACHE:
        _NC_CACHE.append(build_bass())
    nc = _NC_CACHE[0]
    res = run_bass_kernel_spmd(nc, in_maps, core_ids=list(range(8)))
    out = np.zeros((B, S, D), dtype=np.float32)
    for c in range(8):
        out[c // 4] += np.asarray(res.results[c]["out"], dtype=np.float32)
    # V-bias folds into a constant output row: vec_norm += bv  ->  out += bv @ Wo^T
    bvw = np.asarray(bv, dtype=np.float32) @ np.asarray(wo, dtype=np.float32).T
    out += (bvw + np.asarray(bo, dtype=np.float32))[None, None, :]
    return out


# revision 47
# speedup vs baseline: 1.1667x; 1.0469x over previous
"""Multi-head causal attention (B=2,S=2048,D=1024,H=16,RoPE) on 8 TRN2 NeuronCores.

Sharding: core c handles batch b=c//4, head-group g=c%4 (4 heads each).
Wq/Wk/Wv column-sharded per head group, Wo row-sharded; the all-reduce over
head groups is realized as a host-side partial sum at gather time.

Per-core kernel, all matmul operands bf16 (fp32 PSUM accumulation):
  Projection round r: QKV projections for s-block r from pre-transposed x,
    Q/K kept feature-major [d, s], RoPE'd via partition-shift DMAs + DVE
    (pieces split in two so shift DMAs get slack before their consumers);
    V natural [s, d] with a ones column per head (softmax denominators ride
    the AV matmul; the V bias folds into a constant output row added on the
    host: out += bv @ Wo^T).
  Attention (m, qb): per head-pair m the two heads' score matmuls are
    row-tiled (K=64 at partition bases 0/64) into one 2-bank PSUM pair tile
    and run concurrently; a single paired exp [128, 2, 512-c0] on ACT covers
    both heads; binary diag-mask multiply on gpsimd; AV accumulation into
    [65, 512] PSUM per head.
  Weaving: projection round r+1 is emitted as fill pieces inside attention
    round r's kt loop, and output projections are deferred toward the
    ACT-heavy late rounds, so exp-bound attention stretches keep the PE
    busy; reserved boundary pieces cover round-transition drains.
  Normalize (race-hardened, engine-semaphore synced): puv evacuated by ACT
    into a rotating persistent SBUF buffer (frees the PSUM bank fast), DVE
    stream-transpose spreads the denominator row over 32 lanes, reciprocal,
    second transpose recovers the 1/d row, one SBUF->SBUF broadcast DMA
    (stride-0 free dim), and the divide (normalize_b) is deliberately
    emitted a round later so the broadcast has landed long before its
    consumer issues; hp1 halves reach partitions 64-127 via cross-quadrant
    32-partition DVE copies instead of DMAs.
  Output projection: vecT @ Wo per 128-q chunk, bf16 partial out -> DRAM.
  DMA rings: bulk loads + shifts on the sync HWDGE ring (fine granularity so
    no transfer blocks a latency chain), out stores + 1/d broadcasts on the
    scalar ring; the ACT/Pool queues stay compute-only.
"""
import numpy as np
import ml_dtypes
from contextlib import ExitStack

import concourse.bass as bass
import concourse.tile as tile
from concourse import library_config, mybir
from concourse.bass_utils import run_bass_kernel_spmd

B, S, D, H, HD = 2, 2048, 1024, 16, 64
HPC = 4            # heads per core
DC = HPC * HD      # 256 features per core
NDT = D // 128     # 8 input-dim tiles
NST = S // 128     # 16 sequence/key tiles
NQB = S // 512     # 4 query blocks
MT = DC // 128     # 2 feature m-tiles for Q/K/vec

F32 = mybir.dt.float32
BF16 = mybir.dt.bfloat16
AF = mybir.ActivationFunctionType

_nop_ctr = [0]


def fix_engine_waits(nc, max_waits=1):
    """This walrus build rejects any engine instruction with >1 sync wait
    (single wait slot per instruction struct). Move excess waits onto
    same-engine NoOps inserted just before, one wait per NoOp. InstISA is
    skipped (fixed-length encoding)."""
    moved = 0
    for f in nc.m.functions:
        for b in f.blocks:
            insts = b.instructions
            i = 0
            while i < len(insts):
                inst = insts[i]
                if inst.sync_info is not None:
                    # ISA instructions have fixed-length encoding: they can
                    # carry no waits at all, so move every wait to NoOps
                    lim = 0 if inst.opcode == "ISA" else max_waits
                    si = inst.sync_info
                    waits = list(si.on_wait)
                    if len(waits) > lim:
                        keep = waits[len(waits) - lim :] if lim else []
                        for w in waits[: len(waits) - lim]:
                            _nop_ctr[0] += 1
                            moved += 1
                            nop = mybir.InstNoOp(
                                name=f"I-waitnop-{_nop_ctr[0]}", ins=[], outs=[]
                            )
                            nop.engine = inst.engine
                            nop.sync_info = mybir.SyncInfo(on_wait=[w], on_update=[])
                            insts.insert(i, nop)
                            i += 1
                        si.on_wait = keep
                        inst.sync_info = si
                i += 1
    return moved


def _attention_body(ctx: ExitStack, tc, inp, out_ap):
    nc = tc.nc

    persist = ctx.enter_context(tc.tile_pool(name="persist", bufs=1))
    wpool = ctx.enter_context(tc.tile_pool(name="wpool", bufs=1))
    xtp = ctx.enter_context(tc.tile_pool(name="xtp", bufs=2))
    qtmp_p = ctx.enter_context(tc.tile_pool(name="qtmp", bufs=3))
    tsh_p = ctx.enter_context(tc.tile_pool(name="tsh", bufs=3))
    tb2_p = ctx.enter_context(tc.tile_pool(name="tb2", bufs=2))
    exp_p = ctx.enter_context(tc.tile_pool(name="expp", bufs=4))
    rrec_p = ctx.enter_context(tc.tile_pool(name="rrec", bufs=2))
    vtmp_p = ctx.enter_context(tc.tile_pool(name="vtmp", bufs=2))
    tout_p = ctx.enter_context(tc.tile_pool(name="toutp", bufs=2))
    ps_pair = ctx.enter_context(tc.tile_pool(name="ps_pair", bufs=2, space="PSUM"))
    ps_uvec = ctx.enter_context(tc.tile_pool(name="ps_uvec", bufs=2, space="PSUM"))
    ps_work = ctx.enter_context(tc.tile_pool(name="ps_work", bufs=2, space="PSUM"))

    # ---- persistent tensors ----
    qrt = persist.tile([128, MT, S], BF16)      # rotated Q^T  (d-major)
    krt = persist.tile([128, MT, S], BF16)      # rotated K^T
    vext = persist.tile([128, NST, HPC * 65], BF16)  # V tiles + ones col per head
    vecT = persist.tile([128, MT, S], BF16)     # normalized attention output^T
    cos_sb = persist.tile([128, S], BF16)
    sin_sb = persist.tile([128, S], BF16)
    wo_sb = persist.tile([128, MT, D], BF16)
    mdiag_sb = persist.tile([128, 128], BF16)   # binary causal mask, diag block^T
    bq_sb = persist.tile([128, MT], F32)
    uvz = persist.tile([96, 6, 512], BF16)      # rotating evac buffers
    rtz = persist.tile([96, 4, 512], F32)       # rotating strided-recip buffers

    # weights first (per d-tile so the first matmuls start early), then consts
    wq_sb = wpool.tile([128, NDT, DC], BF16)
    wk_sb = wpool.tile([128, NDT, DC], BF16)
    wv_sb = wpool.tile([128, NDT, DC], BF16)
    xT_view = inp["xT"].rearrange("(dt p) s -> p dt s", p=128)
    xts = [
        xtp.tile([128, NDT, 512], BF16, tag="xt", name=f"xt{sb}") for sb in range(2)
    ]
    # startup order matters: everything rides the sync HWDGE ring FIFO, so
    # place each tensor just before its first consumer needs it; the rest of
    # the loads are woven between the round-0 projection pieces below
    for dt in range(NDT):
        nc.sync.dma_start(xts[0][:, dt, :], xT_view[:, dt, 0:512])
        nc.sync.dma_start(
            wq_sb[:, dt, :],
            inp["wqT"].rearrange("(dt p) o -> p dt o", p=128)[:, dt, :],
        )
    nc.sync.dma_start(bq_sb[:, :], inp["bqc"])
    nc.gpsimd.memset(uvz[64:96, :, :], 0.0)
    nc.gpsimd.memset(rtz[64:96, :, :], 0.0)
    nc.sync.dma_start(cos_sb[:, :], inp["cosT"])
    nc.sync.dma_start(sin_sb[:, :], inp["sinT"])

    def load_rest_of_consts(step):
        if step == 0:
            for dt in range(NDT):
                nc.sync.dma_start(
                    wv_sb[:, dt, :],
                    inp["wvT"].rearrange("(dt p) o -> p dt o", p=128)[:, dt, :],
                )
        elif step == 1:
            for dt in range(NDT):
                nc.sync.dma_start(
                    wk_sb[:, dt, :],
                    inp["wkT"].rearrange("(dt p) o -> p dt o", p=128)[:, dt, :],
                )
            nc.sync.dma_start(mdiag_sb[:, :], inp["mdiagT"])
        elif step == 2:
            # ones columns of vext (col 64 of each head slot, every k-tile)
            vones_dst = vext[:, :, :].rearrange("p st (h e) -> p st h e", e=65)[
                :, :, :, 64:65
            ]
            nc.sync.dma_start(
                vones_dst,
                inp["vones"].rearrange("p (st h e) -> p st h e", st=NST, h=HPC),
            )
        elif step == 3:
            for dt in range(NDT):
                nc.sync.dma_start(xts[1][:, dt, :], xT_view[:, dt, 512:1024])
            nc.sync.dma_start(
                wo_sb[:, :, :], inp["woT"].rearrange("(mt p) o -> p mt o", p=128)
            )

    def prefetch_xt(sb):
        xtn = xtp.tile([128, NDT, 512], BF16, tag="xt", name=f"xt{sb}")
        for dt in range(NDT):
            nc.sync.dma_start(
                xtn[:, dt, :], xT_view[:, dt, sb * 512 : (sb + 1) * 512]
            )
        return xtn

    # ---- phase pieces ----
    def qk_proj_piece(m, sb, which, xt):
        """Returns (a, b): a = matmuls + PSUM evac + shift DMAs, b = RoPE
        muls. Emitting b a few fill slots after a gives the shift DMAs slack
        before their consumer issues."""
        ssl = slice(sb * 512, (sb + 1) * 512)
        is_q = which == 0
        dst = qrt if is_q else krt
        w_sb = wq_sb if is_q else wk_sb
        tag_q = "q" if is_q else "k"
        state = {}

        def run_a():
            psq = ps_work.tile([128, 512], F32, tag="w", name=f"psq{tag_q}_{m}_{sb}")
            for dt in range(NDT):
                nc.tensor.matmul(
                    psq[:, :],
                    w_sb[:, dt, m * 128 : (m + 1) * 128],
                    xt[:, dt, :],
                    start=(dt == 0),
                    stop=(dt == NDT - 1),
                )
            qt = qtmp_p.tile([128, 512], BF16, tag="qt", name=f"qt{tag_q}_{m}_{sb}")
            if is_q:
                nc.scalar.activation(
                    qt[:, :], psq[:, :], AF.Identity, bias=bq_sb[:, m : m + 1]
                )
            else:
                nc.scalar.copy(qt[:, :], psq[:, :])
            # rotate_half partition shift p ^ 32 via 4 contiguous DMAs
            sh = tsh_p.tile([128, 512], BF16, tag="sh", name=f"sh{tag_q}_{m}_{sb}")
            for base in (0, 64):
                nc.sync.dma_start(
                    sh[base : base + 32, :], qt[base + 32 : base + 64, :],
                    single_packet=True,
                )
                nc.sync.dma_start(
                    sh[base + 32 : base + 64, :], qt[base : base + 32, :],
                    single_packet=True,
                )
            state["qt"], state["sh"] = qt, sh

        def run_b():
            qt, sh = state["qt"], state["sh"]
            dsl = dst[:, m, ssl]
            tb2 = tb2_p.tile([128, 512], BF16, tag="tb2", name=f"tb2{tag_q}_{m}_{sb}")
            nc.vector.tensor_mul(dsl, qt[:, :], cos_sb[:, ssl])
            nc.vector.tensor_mul(tb2[:, :], sh[:, :], sin_sb[:, ssl])
            nc.vector.tensor_add(dsl, dsl, tb2[:, :])

        return run_a, run_b

    def v_proj_piece(st, xt):
        def run():
            psv = ps_work.tile([128, 512], F32, tag="w", name=f"psv_{st}")[:, 0:256]
            for dt in range(NDT):
                nc.tensor.matmul(
                    psv[:, :],
                    xt[:, dt, (st % 4) * 128 : (st % 4 + 1) * 128],
                    wv_sb[:, dt, :],
                    start=(dt == 0),
                    stop=(dt == NDT - 1),
                )
            vdst = vext[:, st, :].rearrange("p (h e) -> p h e", e=65)[:, :, 0:64]
            nc.vector.tensor_copy(vdst, psv[:, :].rearrange("p (h e) -> p h e", e=64))
        return run

    def outproj_piece(qt_i):
        def run():
            qsl = slice(qt_i * 128, (qt_i + 1) * 128)
            to = tout_p.tile([128, 2, 512], BF16, tag="to", name=f"to_{qt_i}")
            for oc in range(2):
                osl = slice(oc * 512, (oc + 1) * 512)
                pso = ps_work.tile([128, 512], F32, tag="w", name=f"pso_{qt_i}_{oc}")
                for mt in range(MT):
                    nc.tensor.matmul(
                        pso[:, :],
                        vecT[:, mt, qsl],
                        wo_sb[:, mt, osl],
                        start=(mt == 0),
                        stop=(mt == MT - 1),
                    )
                nc.vector.tensor_copy(to[:, oc, :], pso[:, :])
            nc.scalar.dma_start(out_ap[qsl, :], to[:, :, :])
        return run

    _norm_calls = [0]

    def normalize_a(m, hp, qb, puv_t):
        """Evacuate PSUM, compute the broadcast 1/d tile. Returns (uv, rb)."""
        h = 2 * m + hp
        ci = _norm_calls[0]
        _norm_calls[0] += 1
        # evacuate PSUM immediately so the bank frees fast
        # rotating persistent slot; rows 65-95 keep their startup zeros
        # (subtile deps order the reuse after the previous reader)
        uv = uvz[:, ci % 6, :]
        if hp == 0:
            nc.scalar.copy(uv[0:65, :], puv_t[:, :])
        else:
            nc.vector.tensor_copy(uv[0:65, :], puv_t[:, :])
        # spread the 512 denominators over 32 partitions with the DVE
        # stream-square transpose (block t puts d[32t+p] at [p, 32t]) so the
        # reciprocal runs 16 elems/lane instead of 512 on one lane
        dct = rrec_p.tile([96, 512], BF16, tag="dt", name=f"dct_{h}_{qb}", bufs=4)
        nc.vector.transpose(dct[64:96, :], uv[64:96, :])
        dview = dct[64:96, :].rearrange("p (t j) -> p t j", j=32)[:, :, 0:1]
        # reciprocal written strided back into row-major position, second
        # transpose recovers the full 1/d row on one partition — no DRAM
        rt = rtz[:, ci % 4, :]
        rt_view = rt[64:96, :].rearrange("p (t j) -> p t j", j=32)[:, :, 0:1]
        nc.vector.reciprocal(rt_view, dview)
        rt2 = rrec_p.tile([96, 1, 512], F32, tag="r2", name=f"rt2_{h}_{qb}", bufs=4)
        nc.vector.transpose(rt2[64:96, 0, :], rt[64:96, :])
        # partition-broadcast of the 1/d row as one SBUF->SBUF DMA (stride-0
        # on a free dim); its consumer is deliberately emitted much later
        rb = rrec_p.tile([64, 512], F32, tag="rb", name=f"rb_{h}_{qb}", bufs=4)
        nc.sync.dma_start(rb[:, :], rt2[64:65, :, :].to_broadcast([1, 64, 512]))
        return uv, rb

    def normalize_b(m, hp, qb, uv, rb):
        qsl = slice(qb * 512, (qb + 1) * 512)
        h = 2 * m + hp
        if hp == 0:
            nc.vector.tensor_mul(vecT[0:64, m, qsl], uv[0:64, :], rb[:, :])
        else:
            vt = vtmp_p.tile([64, 512], BF16, tag="vt", name=f"vt_{h}_{qb}")
            nc.vector.tensor_mul(vt[:, :], uv[0:64, :], rb[:, :])
            # cross-quadrant DVE copies (32-partition window moves) keep the
            # outproj stationary dependency on engine semaphores, not DMAs
            nc.vector.tensor_copy(vecT[64:96, m, qsl], vt[0:32, :])
            nc.vector.tensor_copy(vecT[96:128, m, qsl], vt[32:64, :])

    # ---- attention with woven fill pieces ----
    fill_queue = []
    pending_nb = []

    def emit_fill(n):
        for _ in range(n):
            if fill_queue:
                fill_queue.pop(0)()

    def attn_qb(m, qb, fill_every):
        puv = [
            ps_uvec.tile([65, 512], F32, tag="u", name=f"puv_m{m}h{hp}q{qb}")
            for hp in range(2)
        ]
        nkt = 4 * qb + 4
        pending = None
        for kt in range(nkt + 1):
            if kt < nkt:
                qb0 = kt // 4
                c0 = (kt % 4) * 128 if qb == qb0 else 0
                psc = ps_pair.tile(
                    [128, 2, 512], F32, tag="pair", name=f"psc_m{m}q{qb}k{kt}"
                )
                for hp in range(2):
                    pb = hp * 64
                    nc.tensor.matmul(
                        psc[:, hp, c0:512],
                        krt[pb : pb + 64, m, kt * 128 : (kt + 1) * 128],
                        qrt[pb : pb + 64, m, qb * 512 + c0 : (qb + 1) * 512],
                        start=True,
                        stop=True,
                    )
                et = exp_p.tile(
                    [128, 2, 512], BF16, tag="e", name=f"et_m{m}q{qb}k{kt}"
                )
                nc.scalar.activation(
                    et[:, :, c0:512], psc[:, :, c0:512], AF.Exp, scale=0.125
                )
                if qb == qb0:
                    for hp in range(2):
                        nc.gpsimd.tensor_mul(
                            et[:, hp, c0 : c0 + 128],
                            et[:, hp, c0 : c0 + 128],
                            mdiag_sb[:, :],
                        )
                cur = (kt, c0, et)
            else:
                cur = None
            if pending is not None:
                pkt, pc0, pet = pending
                for hp in range(2):
                    h = 2 * m + hp
                    nc.tensor.matmul(
                        puv[hp][:, pc0:512],
                        vext[:, pkt, h * 65 : (h + 1) * 65],
                        pet[:, hp, pc0:512],
                        start=(pkt == 0),
                        stop=(pkt == nkt - 1),
                        skip_group_check=True,
                    )
            pending = cur
            if fill_every and (kt % fill_every == fill_every - 1):
                emit_fill(1)
        for hp in range(2):
            uv, rb = normalize_a(m, hp, qb, puv[hp])
            pending_nb.append(
                lambda m=m, hp=hp, qb=qb, uv=uv, rb=rb: normalize_b(
                    m, hp, qb, uv, rb
                )
            )

    # round 0 projections standalone, const loads woven between pieces so no
    # single bulk DMA delays the round-0 shift DMAs in the ring FIFO
    qa0, qb0 = qk_proj_piece(0, 0, 0, xts[0])
    qa1, qb1 = qk_proj_piece(1, 0, 0, xts[0])
    qa0(); qa1()
    load_rest_of_consts(0)
    v_proj_piece(0, xts[0])()
    v_proj_piece(1, xts[0])()
    qb0(); qb1()
    load_rest_of_consts(1)
    ka0, kb0 = qk_proj_piece(0, 0, 1, xts[0])
    ka1, kb1 = qk_proj_piece(1, 0, 1, xts[0])
    ka0(); ka1()
    load_rest_of_consts(2)
    v_proj_piece(2, xts[0])()
    v_proj_piece(3, xts[0])()
    kb0(); kb1()
    load_rest_of_consts(3)

    for r in range(NQB):
        if r + 2 < NQB:
            xts.append(prefetch_xt(r + 2))
        # normalize division for round r-1, emitted a round late so the 1/d
        # broadcast DMAs have long since landed
        for nb_fn in pending_nb:
            nb_fn()
        pending_nb.clear()
        pieces = []
        boundary = []
        if r + 1 < NQB:
            xt_n = xts[r + 1]
            nqa0, nqb0 = qk_proj_piece(0, r + 1, 0, xt_n)
            nqa1, nqb1 = qk_proj_piece(1, r + 1, 0, xt_n)
            nka0, nkb0 = qk_proj_piece(0, r + 1, 1, xt_n)
            nka1, nkb1 = qk_proj_piece(1, r + 1, 1, xt_n)
            pieces += [
                nqa0,
                nqa1,
                v_proj_piece(4 * (r + 1) + 0, xt_n),
                v_proj_piece(4 * (r + 1) + 1, xt_n),
                nqb0,
                nqb1,
                nka0,
                nka1,
                v_proj_piece(4 * (r + 1) + 2, xt_n),
                nkb0,
                nkb1,
            ]
            # last V tile of the next round is not needed until deep into
            # round r+1 — hold it back to cover the round-boundary drain
            boundary.append(v_proj_piece(4 * (r + 1) + 3, xt_n))
        # outproj work is deferred toward the ACT-heavy late rounds: round 3
        # has the largest exp load and the least projection fill left
        if r == 1:
            boundary = [outproj_piece(0), outproj_piece(1)] + boundary
        elif r == 2:
            pieces = pieces + [outproj_piece(4), outproj_piece(5)]
            boundary = [outproj_piece(2), outproj_piece(3)] + boundary
        elif r == 3:
            pieces = pieces + [outproj_piece(i) for i in range(6, 12)]
        fill_queue.extend(pieces)
        nkts = 2 * (4 * r + 4)
        fill_every = max(1, nkts // (len(fill_queue) + 1)) if fill_queue else 0
        attn_qb(0, r, fill_every)
        attn_qb(1, r, fill_every)
        emit_fill(len(fill_queue))
        for piece in boundary:
            piece()

    for nb_fn in pending_nb:
        nb_fn()
    pending_nb.clear()
    for i in range(4):
        outproj_piece(12 + i)()


def build_bass(fix_waits=True):
    nc = bass.Bass("TRN2", debug=False)
    inp = {}

    def din(name, shape, dtype=BF16):
        inp[name] = nc.dram_tensor(name, list(shape), dtype, kind="ExternalInput").ap()

    din("xT", (D, S))
    din("wqT", (D, DC))
    din("wkT", (D, DC))
    din("wvT", (D, DC))
    din("bqc", (128, MT), F32)
    din("cosT", (128, S))
    din("sinT", (128, S))
    din("mdiagT", (128, 128))
    din("woT", (DC, D))
    din("vones", (128, NST * HPC))
    out_ap = nc.dram_tensor("out", [S, D], BF16, kind="ExternalOutput").ap()

    with tile.TileContext(nc) as tc:
        with ExitStack() as ctx:
            _attention_body(ctx, tc, inp, out_ap)
    if fix_waits:
        fix_engine_waits(nc)
    return nc


# ---- host-side sharding / prep ----


def make_core_inputs(x, mask, cos, sin, wq, bq, wk, wv, bv, wo):
    """Returns list of 8 input dicts (core c = batch c//4, head-group c%4)."""
    bf16 = ml_dtypes.bfloat16
    x = np.ascontiguousarray(x, dtype=np.float32)
    p = np.arange(128)
    pf = p % 64
    cosT = np.ascontiguousarray(cos.T[pf, :]).astype(bf16)          # [128, S]
    sgn = np.where(pf < 32, -1.0, 1.0).astype(np.float32)
    sinT = np.ascontiguousarray(sgn[:, None] * sin.T[pf, :]).astype(bf16)
    mdiagT = np.ascontiguousarray(
        (mask[0:128, 0:128].T == 0).astype(np.float32)
    ).astype(bf16)
    vones = np.ones((128, NST * HPC), dtype=bf16)

    in_maps = []
    for c in range(8):
        b, g = c // 4, c % 4
        rows = np.arange(g * DC, (g + 1) * DC)
        bqc = np.ascontiguousarray(bq[rows].reshape(MT, 128).T, dtype=np.float32)
        in_maps.append({
            "xT": np.ascontiguousarray(x[b].T).astype(bf16),
            "wqT": np.ascontiguousarray(wq[rows].T).astype(bf16),
            "wkT": np.ascontiguousarray(wk[rows].T).astype(bf16),
            "wvT": np.ascontiguousarray(wv[rows].T).astype(bf16),
            "bqc": bqc,
            "cosT": cosT,
            "sinT": sinT,
            "mdiagT": mdiagT,
            "woT": np.ascontiguousarray(wo[:, rows].T).astype(bf16),
            "vones": vones,
        })
    return in_maps


_NC_CACHE = []


def kernel(x, mask, cos, sin, wq, bq, wk, wv, bv, wo, bo):
    x = np.asarray(x, dtype=np.float32)
    in_maps = make_core_inputs(
        x, np.asarray(mask), np.asarray(cos), np.asarray(sin),
        np.asarray(wq), np.asarray(bq), np.asarray(wk), np.asarray(wv),
        np.asarray(bv), np.asarray(wo),
    )
    if not _NC_CACHE:
        _NC_CACHE.append(build_bass())
    nc = _NC_CACHE[0]
    res = run_bass_kernel_spmd(nc, in_maps, core_ids=list(range(8)))
    out = np.zeros((B, S, D), dtype=np.float32)
    for c in range(8):
        out[c // 4] += np.asarray(res.results[c]["out"], dtype=np.float32)
    # V-bias folds into a constant output row: vec_norm += bv  ->  out += bv @ Wo^T
    bvw = np.asarray(bv, dtype=np.float32) @ np.asarray(wo, dtype=np.float32).T
    out += (bvw + np.asarray(bo, dtype=np.float32))[None, None, :]
    return out


# revision 49
# speedup vs baseline: 1.2010x; 1.0294x over previous
"""Multi-head causal attention (B=2,S=2048,D=1024,H=16,RoPE) on 8 TRN2 NeuronCores.

Sharding: core c handles batch b=c//4, head-group g=c%4 (4 heads each).
Wq/Wk/Wv column-sharded per head group, Wo row-sharded; the all-reduce over
head groups is realized as a host-side partial sum at gather time.

Per-core kernel, all matmul operands bf16 (fp32 PSUM accumulation):
  Projection round r: QKV projections for s-block r from pre-transposed x,
    Q/K kept feature-major [d, s], RoPE'd via partition-shift DMAs + DVE
    (pieces split in two so shift DMAs get slack before their consumers);
    V natural [s, d] with a ones column per head (softmax denominators ride
    the AV matmul; the V bias folds into a constant output row added on the
    host: out += bv @ Wo^T).
  Attention (m, qb): per head-pair m the two heads' score matmuls are
    row-tiled (K=64 at partition bases 0/64) into one 2-bank PSUM pair tile
    and run concurrently; a single paired exp [128, 2, 512-c0] on ACT covers
    both heads; binary diag-mask multiply on gpsimd; AV accumulation into
    [65, 512] PSUM per head.
  Weaving: projection round r+1 is emitted as fill pieces inside attention
    round r's kt loop, and output projections are deferred toward the
    ACT-heavy late rounds, so exp-bound attention stretches keep the PE
    busy; reserved boundary pieces cover round-transition drains.
  Normalize (race-hardened, engine-semaphore synced): puv evacuated by ACT
    into a rotating persistent SBUF buffer (frees the PSUM bank fast), DVE
    stream-transpose spreads the denominator row over 32 lanes, reciprocal,
    second transpose recovers the 1/d row, one SBUF->SBUF broadcast DMA
    (stride-0 free dim), and the divide (normalize_b) is deliberately
    emitted a round later so the broadcast has landed long before its
    consumer issues; hp1 halves reach partitions 64-127 via cross-quadrant
    32-partition DVE copies instead of DMAs.
  Output projection: vecT @ Wo per 128-q chunk, bf16 partial out -> DRAM.
  DMA rings: bulk loads + shifts + 1/d broadcasts on the sync HWDGE ring
    (fine granularity so no transfer blocks a latency chain), out stores on
    the scalar ring; latency-critical dispatches never queue behind the exp
    backlog in the ACT queue.
"""
import numpy as np
import ml_dtypes
from contextlib import ExitStack

import concourse.bass as bass
import concourse.tile as tile
from concourse import library_config, mybir
from concourse.bass_utils import run_bass_kernel_spmd

B, S, D, H, HD = 2, 2048, 1024, 16, 64
HPC = 4            # heads per core
DC = HPC * HD      # 256 features per core
NDT = D // 128     # 8 input-dim tiles
NST = S // 128     # 16 sequence/key tiles
NQB = S // 512     # 4 query blocks
MT = DC // 128     # 2 feature m-tiles for Q/K/vec

F32 = mybir.dt.float32
BF16 = mybir.dt.bfloat16
AF = mybir.ActivationFunctionType

_nop_ctr = [0]


def fix_engine_waits(nc, max_waits=1):
    """This walrus build rejects any engine instruction with >1 sync wait
    (single wait slot per instruction struct). Move excess waits onto
    same-engine NoOps inserted just before, one wait per NoOp. InstISA is
    skipped (fixed-length encoding)."""
    moved = 0
    for f in nc.m.functions:
        for b in f.blocks:
            insts = b.instructions
            i = 0
            while i < len(insts):
                inst = insts[i]
                if inst.sync_info is not None:
                    # ISA instructions have fixed-length encoding: they can
                    # carry no waits at all, so move every wait to NoOps
                    lim = 0 if inst.opcode == "ISA" else max_waits
                    si = inst.sync_info
                    waits = list(si.on_wait)
                    if len(waits) > lim:
                        keep = waits[len(waits) - lim :] if lim else []
                        for w in waits[: len(waits) - lim]:
                            _nop_ctr[0] += 1
                            moved += 1
                            nop = mybir.InstNoOp(
                                name=f"I-waitnop-{_nop_ctr[0]}", ins=[], outs=[]
                            )
                            nop.engine = inst.engine
                            nop.sync_info = mybir.SyncInfo(on_wait=[w], on_update=[])
                            insts.insert(i, nop)
                            i += 1
                        si.on_wait = keep
                        inst.sync_info = si
                i += 1
    return moved


def _attention_body(ctx: ExitStack, tc, inp, out_ap):
    nc = tc.nc

    persist = ctx.enter_context(tc.tile_pool(name="persist", bufs=1))
    wpool = ctx.enter_context(tc.tile_pool(name="wpool", bufs=1))
    xtp = ctx.enter_context(tc.tile_pool(name="xtp", bufs=2))
    qtmp_p = ctx.enter_context(tc.tile_pool(name="qtmp", bufs=3))
    tsh_p = ctx.enter_context(tc.tile_pool(name="tsh", bufs=3))
    tb2_p = ctx.enter_context(tc.tile_pool(name="tb2", bufs=2))
    exp_p = ctx.enter_context(tc.tile_pool(name="expp", bufs=4))
    rrec_p = ctx.enter_context(tc.tile_pool(name="rrec", bufs=2))
    vtmp_p = ctx.enter_context(tc.tile_pool(name="vtmp", bufs=2))
    tout_p = ctx.enter_context(tc.tile_pool(name="toutp", bufs=2))
    ps_pair = ctx.enter_context(tc.tile_pool(name="ps_pair", bufs=2, space="PSUM"))
    ps_uvec = ctx.enter_context(tc.tile_pool(name="ps_uvec", bufs=2, space="PSUM"))
    ps_work = ctx.enter_context(tc.tile_pool(name="ps_work", bufs=2, space="PSUM"))

    # ---- persistent tensors ----
    qrt = persist.tile([128, MT, S], BF16)      # rotated Q^T  (d-major)
    krt = persist.tile([128, MT, S], BF16)      # rotated K^T
    vext = persist.tile([128, NST, HPC * 65], BF16)  # V tiles + ones col per head
    vecT = persist.tile([128, MT, S], BF16)     # normalized attention output^T
    cos_sb = persist.tile([128, S], BF16)
    sin_sb = persist.tile([128, S], BF16)
    wo_sb = persist.tile([128, MT, D], BF16)
    mdiag_sb = persist.tile([128, 128], BF16)   # binary causal mask, diag block^T
    bq_sb = persist.tile([128, MT], F32)
    uvz = persist.tile([96, 6, 512], BF16)      # rotating evac buffers
    rtz = persist.tile([96, 4, 512], F32)       # rotating strided-recip buffers

    # weights first (per d-tile so the first matmuls start early), then consts
    wq_sb = wpool.tile([128, NDT, DC], BF16)
    wk_sb = wpool.tile([128, NDT, DC], BF16)
    wv_sb = wpool.tile([128, NDT, DC], BF16)
    xT_view = inp["xT"].rearrange("(dt p) s -> p dt s", p=128)
    xts = [
        xtp.tile([128, NDT, 512], BF16, tag="xt", name=f"xt{sb}") for sb in range(2)
    ]
    # startup order matters: everything rides the sync HWDGE ring FIFO, so
    # place each tensor just before its first consumer needs it; the rest of
    # the loads are woven between the round-0 projection pieces below
    for dt in range(NDT):
        nc.sync.dma_start(xts[0][:, dt, :], xT_view[:, dt, 0:512])
        nc.scalar.dma_start(
            wq_sb[:, dt, :],
            inp["wqT"].rearrange("(dt p) o -> p dt o", p=128)[:, dt, :],
        )
    nc.sync.dma_start(bq_sb[:, :], inp["bqc"])
    nc.gpsimd.memset(uvz[64:96, :, :], 0.0)
    nc.gpsimd.memset(rtz[64:96, :, :], 0.0)
    nc.sync.dma_start(cos_sb[:, :], inp["cosT"])
    nc.sync.dma_start(sin_sb[:, :], inp["sinT"])

    def load_rest_of_consts(step):
        if step == 0:
            for dt in range(NDT):
                nc.scalar.dma_start(
                    wv_sb[:, dt, :],
                    inp["wvT"].rearrange("(dt p) o -> p dt o", p=128)[:, dt, :],
                )
        elif step == 1:
            for dt in range(NDT):
                nc.scalar.dma_start(
                    wk_sb[:, dt, :],
                    inp["wkT"].rearrange("(dt p) o -> p dt o", p=128)[:, dt, :],
                )
            nc.sync.dma_start(mdiag_sb[:, :], inp["mdiagT"])
        elif step == 2:
            # ones columns of vext (col 64 of each head slot, every k-tile)
            vones_dst = vext[:, :, :].rearrange("p st (h e) -> p st h e", e=65)[
                :, :, :, 64:65
            ]
            nc.sync.dma_start(
                vones_dst,
                inp["vones"].rearrange("p (st h e) -> p st h e", st=NST, h=HPC),
            )
        elif step == 3:
            for dt in range(NDT):
                nc.sync.dma_start(xts[1][:, dt, :], xT_view[:, dt, 512:1024])
            nc.sync.dma_start(
                wo_sb[:, :, :], inp["woT"].rearrange("(mt p) o -> p mt o", p=128)
            )

    def prefetch_xt(sb):
        xtn = xtp.tile([128, NDT, 512], BF16, tag="xt", name=f"xt{sb}")
        for dt in range(NDT):
            nc.sync.dma_start(
                xtn[:, dt, :], xT_view[:, dt, sb * 512 : (sb + 1) * 512]
            )
        return xtn

    # ---- phase pieces ----
    def qk_proj_piece(m, sb, which, xt):
        """Returns (a, b): a = matmuls + PSUM evac + shift DMAs, b = RoPE
        muls. Emitting b a few fill slots after a gives the shift DMAs slack
        before their consumer issues."""
        ssl = slice(sb * 512, (sb + 1) * 512)
        is_q = which == 0
        dst = qrt if is_q else krt
        w_sb = wq_sb if is_q else wk_sb
        tag_q = "q" if is_q else "k"
        state = {}

        def run_a():
            psq = ps_work.tile([128, 512], F32, tag="w", name=f"psq{tag_q}_{m}_{sb}")
            for dt in range(NDT):
                nc.tensor.matmul(
                    psq[:, :],
                    w_sb[:, dt, m * 128 : (m + 1) * 128],
                    xt[:, dt, :],
                    start=(dt == 0),
                    stop=(dt == NDT - 1),
                )
            qt = qtmp_p.tile([128, 512], BF16, tag="qt", name=f"qt{tag_q}_{m}_{sb}")
            if is_q:
                nc.scalar.activation(
                    qt[:, :], psq[:, :], AF.Identity, bias=bq_sb[:, m : m + 1]
                )
            else:
                nc.scalar.copy(qt[:, :], psq[:, :])
            # rotate_half partition shift p ^ 32 via 4 contiguous DMAs
            sh = tsh_p.tile([128, 512], BF16, tag="sh", name=f"sh{tag_q}_{m}_{sb}")
            for base in (0, 64):
                nc.sync.dma_start(
                    sh[base : base + 32, :], qt[base + 32 : base + 64, :],
                    single_packet=True,
                )
                nc.sync.dma_start(
                    sh[base + 32 : base + 64, :], qt[base : base + 32, :],
                    single_packet=True,
                )
            state["qt"], state["sh"] = qt, sh

        def run_b():
            qt, sh = state["qt"], state["sh"]
            dsl = dst[:, m, ssl]
            tb2 = tb2_p.tile([128, 512], BF16, tag="tb2", name=f"tb2{tag_q}_{m}_{sb}")
            nc.vector.tensor_mul(dsl, qt[:, :], cos_sb[:, ssl])
            nc.vector.tensor_mul(tb2[:, :], sh[:, :], sin_sb[:, ssl])
            nc.vector.tensor_add(dsl, dsl, tb2[:, :])

        return run_a, run_b

    def v_proj_piece(st, xt):
        def run():
            psv = ps_work.tile([128, 512], F32, tag="w", name=f"psv_{st}")[:, 0:256]
            for dt in range(NDT):
                nc.tensor.matmul(
                    psv[:, :],
                    xt[:, dt, (st % 4) * 128 : (st % 4 + 1) * 128],
                    wv_sb[:, dt, :],
                    start=(dt == 0),
                    stop=(dt == NDT - 1),
                )
            vdst = vext[:, st, :].rearrange("p (h e) -> p h e", e=65)[:, :, 0:64]
            nc.vector.tensor_copy(vdst, psv[:, :].rearrange("p (h e) -> p h e", e=64))
        return run

    def outproj_piece(qt_i):
        def run():
            qsl = slice(qt_i * 128, (qt_i + 1) * 128)
            to = tout_p.tile([128, 2, 512], BF16, tag="to", name=f"to_{qt_i}")
            for oc in range(2):
                osl = slice(oc * 512, (oc + 1) * 512)
                pso = ps_work.tile([128, 512], F32, tag="w", name=f"pso_{qt_i}_{oc}")
                for mt in range(MT):
                    nc.tensor.matmul(
                        pso[:, :],
                        vecT[:, mt, qsl],
                        wo_sb[:, mt, osl],
                        start=(mt == 0),
                        stop=(mt == MT - 1),
                    )
                nc.vector.tensor_copy(to[:, oc, :], pso[:, :])
            nc.scalar.dma_start(out_ap[qsl, :], to[:, :, :])
        return run

    _norm_calls = [0]

    def normalize_a(m, hp, qb, puv_t):
        """Evacuate PSUM, compute the broadcast 1/d tile. Returns (uv, rb)."""
        h = 2 * m + hp
        ci = _norm_calls[0]
        _norm_calls[0] += 1
        # evacuate PSUM immediately so the bank frees fast
        # rotating persistent slot; rows 65-95 keep their startup zeros
        # (subtile deps order the reuse after the previous reader)
        uv = uvz[:, ci % 6, :]
        if hp == 0:
            nc.scalar.copy(uv[0:65, :], puv_t[:, :])
        else:
            nc.vector.tensor_copy(uv[0:65, :], puv_t[:, :])
        # spread the 512 denominators over 32 partitions with the DVE
        # stream-square transpose (block t puts d[32t+p] at [p, 32t]) so the
        # reciprocal runs 16 elems/lane instead of 512 on one lane
        dct = rrec_p.tile([96, 512], BF16, tag="dt", name=f"dct_{h}_{qb}", bufs=4)
        nc.vector.transpose(dct[64:96, :], uv[64:96, :])
        dview = dct[64:96, :].rearrange("p (t j) -> p t j", j=32)[:, :, 0:1]
        # reciprocal written strided back into row-major position, second
        # transpose recovers the full 1/d row on one partition — no DRAM
        rt = rtz[:, ci % 4, :]
        rt_view = rt[64:96, :].rearrange("p (t j) -> p t j", j=32)[:, :, 0:1]
        nc.vector.reciprocal(rt_view, dview)
        rt2 = rrec_p.tile([96, 1, 512], F32, tag="r2", name=f"rt2_{h}_{qb}", bufs=4)
        nc.vector.transpose(rt2[64:96, 0, :], rt[64:96, :])
        # partition-broadcast of the 1/d row as one SBUF->SBUF DMA (stride-0
        # on a free dim); its consumer is deliberately emitted much later
        rb = rrec_p.tile([64, 512], F32, tag="rb", name=f"rb_{h}_{qb}", bufs=4)
        nc.sync.dma_start(rb[:, :], rt2[64:65, :, :].to_broadcast([1, 64, 512]))
        return uv, rb

    def normalize_b(m, hp, qb, uv, rb):
        qsl = slice(qb * 512, (qb + 1) * 512)
        h = 2 * m + hp
        if hp == 0:
            nc.vector.tensor_mul(vecT[0:64, m, qsl], uv[0:64, :], rb[:, :])
        else:
            vt = vtmp_p.tile([64, 512], BF16, tag="vt", name=f"vt_{h}_{qb}")
            nc.vector.tensor_mul(vt[:, :], uv[0:64, :], rb[:, :])
            # cross-quadrant DVE copies (32-partition window moves) keep the
            # outproj stationary dependency on engine semaphores, not DMAs
            nc.vector.tensor_copy(vecT[64:96, m, qsl], vt[0:32, :])
            nc.vector.tensor_copy(vecT[96:128, m, qsl], vt[32:64, :])

    # ---- attention with woven fill pieces ----
    fill_queue = []
    pending_nb = []

    def emit_fill(n):
        for _ in range(n):
            if fill_queue:
                fill_queue.pop(0)()

    def attn_qb(m, qb, fill_every):
        puv = [
            ps_uvec.tile([65, 512], F32, tag="u", name=f"puv_m{m}h{hp}q{qb}")
            for hp in range(2)
        ]
        nkt = 4 * qb + 4
        pending = None
        for kt in range(nkt + 1):
            if kt < nkt:
                qb0 = kt // 4
                c0 = (kt % 4) * 128 if qb == qb0 else 0
                psc = ps_pair.tile(
                    [128, 2, 512], F32, tag="pair", name=f"psc_m{m}q{qb}k{kt}"
                )
                for hp in range(2):
                    pb = hp * 64
                    nc.tensor.matmul(
                        psc[:, hp, c0:512],
                        krt[pb : pb + 64, m, kt * 128 : (kt + 1) * 128],
                        qrt[pb : pb + 64, m, qb * 512 + c0 : (qb + 1) * 512],
                        start=True,
                        stop=True,
                    )
                et = exp_p.tile(
                    [128, 2, 512], BF16, tag="e", name=f"et_m{m}q{qb}k{kt}"
                )
                nc.scalar.activation(
                    et[:, :, c0:512], psc[:, :, c0:512], AF.Exp, scale=0.125
                )
                if qb == qb0:
                    for hp in range(2):
                        nc.gpsimd.tensor_mul(
                            et[:, hp, c0 : c0 + 128],
                            et[:, hp, c0 : c0 + 128],
                            mdiag_sb[:, :],
                        )
                cur = (kt, c0, et)
            else:
                cur = None
            if pending is not None:
                pkt, pc0, pet = pending
                for hp in range(2):
                    h = 2 * m + hp
                    nc.tensor.matmul(
                        puv[hp][:, pc0:512],
                        vext[:, pkt, h * 65 : (h + 1) * 65],
                        pet[:, hp, pc0:512],
                        start=(pkt == 0),
                        stop=(pkt == nkt - 1),
                        skip_group_check=True,
                    )
            pending = cur
            if fill_every and (kt % fill_every == fill_every - 1):
                emit_fill(1)
        for hp in range(2):
            uv, rb = normalize_a(m, hp, qb, puv[hp])
            pending_nb.append(
                lambda m=m, hp=hp, qb=qb, uv=uv, rb=rb: normalize_b(
                    m, hp, qb, uv, rb
                )
            )

    # round 0 projections standalone, const loads woven between pieces so no
    # single bulk DMA delays the round-0 shift DMAs in the ring FIFO
    qa0, qb0 = qk_proj_piece(0, 0, 0, xts[0])
    qa1, qb1 = qk_proj_piece(1, 0, 0, xts[0])
    qa0(); qa1()
    load_rest_of_consts(0)
    v_proj_piece(0, xts[0])()
    v_proj_piece(1, xts[0])()
    qb0(); qb1()
    load_rest_of_consts(1)
    ka0, kb0 = qk_proj_piece(0, 0, 1, xts[0])
    ka1, kb1 = qk_proj_piece(1, 0, 1, xts[0])
    ka0(); ka1()
    load_rest_of_consts(2)
    v_proj_piece(2, xts[0])()
    v_proj_piece(3, xts[0])()
    kb0(); kb1()
    load_rest_of_consts(3)

    for r in range(NQB):
        if r + 2 < NQB:
            xts.append(prefetch_xt(r + 2))
        # normalize division for round r-1, emitted a round late so the 1/d
        # broadcast DMAs have long since landed
        for nb_fn in pending_nb:
            nb_fn()
        pending_nb.clear()
        pieces = []
        boundary = []
        if r + 1 < NQB:
            xt_n = xts[r + 1]
            nqa0, nqb0 = qk_proj_piece(0, r + 1, 0, xt_n)
            nqa1, nqb1 = qk_proj_piece(1, r + 1, 0, xt_n)
            nka0, nkb0 = qk_proj_piece(0, r + 1, 1, xt_n)
            nka1, nkb1 = qk_proj_piece(1, r + 1, 1, xt_n)
            pieces += [
                nqa0,
                nqa1,
                v_proj_piece(4 * (r + 1) + 0, xt_n),
                v_proj_piece(4 * (r + 1) + 1, xt_n),
                nqb0,
                nqb1,
                nka0,
                nka1,
                v_proj_piece(4 * (r + 1) + 2, xt_n),
                nkb0,
                nkb1,
            ]
            # last V tile of the next round is not needed until deep into
            # round r+1 — hold it back to cover the round-boundary drain
            boundary.append(v_proj_piece(4 * (r + 1) + 3, xt_n))
        # outproj work is deferred toward the ACT-heavy late rounds: round 3
        # has the largest exp load and the least projection fill left
        if r == 1:
            boundary = [outproj_piece(0), outproj_piece(1)] + boundary
        elif r == 2:
            pieces = pieces + [outproj_piece(4), outproj_piece(5)]
            boundary = [outproj_piece(2), outproj_piece(3)] + boundary
        elif r == 3:
            pieces = pieces + [outproj_piece(i) for i in range(6, 12)]
        fill_queue.extend(pieces)
        nkts = 2 * (4 * r + 4)
        fill_every = max(1, nkts // (len(fill_queue) + 1)) if fill_queue else 0
        attn_qb(0, r, fill_every)
        if r == NQB - 1:
            for nb_fn in pending_nb:
                nb_fn()
            pending_nb.clear()
        attn_qb(1, r, fill_every)
        emit_fill(len(fill_queue))
        for piece in boundary:
            piece()

    for nb_fn in pending_nb:
        nb_fn()
    pending_nb.clear()
    for i in range(4):
        outproj_piece(12 + i)()


def build_bass(fix_waits=True):
    nc = bass.Bass("TRN2", debug=False)
    inp = {}

    def din(name, shape, dtype=BF16):
        inp[name] = nc.dram_tensor(name, list(shape), dtype, kind="ExternalInput").ap()

    din("xT", (D, S))
    din("wqT", (D, DC))
    din("wkT", (D, DC))
    din("wvT", (D, DC))
    din("bqc", (128, MT), F32)
    din("cosT", (128, S))
    din("sinT", (128, S))
    din("mdiagT", (128, 128))
    din("woT", (DC, D))
    din("vones", (128, NST * HPC))
    out_ap = nc.dram_tensor("out", [S, D], BF16, kind="ExternalOutput").ap()

    with tile.TileContext(nc) as tc:
        with ExitStack() as ctx:
            _attention_body(ctx, tc, inp, out_ap)
    if fix_waits:
        fix_engine_waits(nc)
    return nc


# ---- host-side sharding / prep ----


def make_core_inputs(x, mask, cos, sin, wq, bq, wk, wv, bv, wo):
    """Returns list of 8 input dicts (core c = batch c//4, head-group c%4)."""
    bf16 = ml_dtypes.bfloat16
    x = np.ascontiguousarray(x, dtype=np.float32)
    p = np.arange(128)
    pf = p % 64
    cosT = np.ascontiguousarray(cos.T[pf, :]).astype(bf16)          # [128, S]
    sgn = np.where(pf < 32, -1.0, 1.0).astype(np.float32)
    sinT = np.ascontiguousarray(sgn[:, None] * sin.T[pf, :]).astype(bf16)
    mdiagT = np.ascontiguousarray(
        (mask[0:128, 0:128].T == 0).astype(np.float32)
    ).astype(bf16)
    vones = np.ones((128, NST * HPC), dtype=bf16)

    in_maps = []
    for c in range(8):
        b, g = c // 4, c % 4
        rows = np.arange(g * DC, (g + 1) * DC)
        bqc = np.ascontiguousarray(bq[rows].reshape(MT, 128).T, dtype=np.float32)
        in_maps.append({
            "xT": np.ascontiguousarray(x[b].T).astype(bf16),
            "wqT": np.ascontiguousarray(wq[rows].T).astype(bf16),
            "wkT": np.ascontiguousarray(wk[rows].T).astype(bf16),
            "wvT": np.ascontiguousarray(wv[rows].T).astype(bf16),
            "bqc": bqc,
            "cosT": cosT,
            "sinT": sinT,
            "mdiagT": mdiagT,
            "woT": np.ascontiguousarray(wo[:, rows].T).astype(bf16),
            "vones": vones,
        })
    return in_maps


_NC_CACHE = []


def kernel(x, mask, cos, sin, wq, bq, wk, wv, bv, wo, bo):
    x = np.asarray(x, dtype=np.float32)
    in_maps = make_core_inputs(
        x, np.asarray(mask), np.asarray(cos), np.asarray(sin),
        np.asarray(wq), np.asarray(bq), np.asarray(wk), np.asarray(wv),
        np.asarray(bv), np.asarray(wo),
    )
    if not _NC_CACHE:
        _NC_CACHE.append(build_bass())
    nc = _NC_CACHE[0]
    res = run_bass_kernel_spmd(nc, in_maps, core_ids=list(range(8)))
    out = np.zeros((B, S, D), dtype=np.float32)
    for c in range(8):
        out[c // 4] += np.asarray(res.results[c]["out"], dtype=np.float32)
    # V-bias folds into a constant output row: vec_norm += bv  ->  out += bv @ Wo^T
    bvw = np.asarray(bv, dtype=np.float32) @ np.asarray(wo, dtype=np.float32).T
    out += (bvw + np.asarray(bo, dtype=np.float32))[None, None, :]
    return out
